# revision 40
# baseline (speedup 1.0000x reference)
"""Causal single-head attention (B=4, S=4096, D=1024, H=128) on 8 NeuronCores.

Sharding: core c = (batch b = c//2, half h = c%2). One shared SPMD program;
all per-half differences are carried in the DATA:
  - host pair-swaps adjacent 128-col blocks of x^T for h=1 cores, so a fixed
    even-local-block gather pattern selects that half's parity-interleaved
    q subtiles (global subtile g = 8r+2s+h for slot r, s in 0..3),
  - kio holds h-shifted global k positions; qpos is generated on-chip by
    iota, so is_ge(qpos, kio) is the exact causal mask.

Each core: K^T/V (full 4096 keys, replicated within the batch pair) + its own
2048 query rows.  Algebraic prunes: bk dropped (softmax shift-invariance along
k), bv folded into a host-side constant add (sum_k att = 1/sqrt(H)).

Pipeline per core (bf16 matmuls, fp32 PSUM):
  x^T host-transposed bf16, loaded as a few block DMAs with 1-2KB lines;
  weights host-preshuffled to [128, 8*128] for 2KB-line loads
  P-phases b=0..3: K/V/Q projections of stripe pair (2b,2b+1), stripe-major;
  V natural via PE transpose; Q gathered straight from resident x^T
  (strided AP); PSUM->SBUF copies ride the ACT engine
  attention in two passes, kt-outer, software-pipelined 2 deep; per kt ONE
  fused exp over both slots' adjacent PSUM banks ([P,2,512] tile);
  single-slot stretches fuse kt PAIRS into one exp.
    pass A = slots (0,1), kts 0..15, P-phases 2+3 interleaved as PE filler
    pass B = slots (2,3), kts 0..31, epilogues of slots 0,1,2 interleaved
  causal mask: one 128-col DVE is_ge STT per kt (only the subtile at the
  c0 boundary straddles the diagonal); denominator in bf16 on DVE with an
  extra accumulator per pass for the fused-pair second kt
  epilogue per slot: d^T by one-col matmuls (dacc as weights), reciprocal,
  O^T -> O by PE transpose, scale by rec*(1/sqrt(H)) via STT, bf16 DMA out
"""

import numpy as np
import ml_dtypes
from contextlib import ExitStack

import concourse.bass as bass
import concourse.tile as tile
from concourse import bacc, mybir
from concourse.bass_utils import run_bass_kernel_spmd

B, S, D, H = 4, 4096, 1024, 128
P = 128
BF16 = mybir.dt.bfloat16
F32 = mybir.dt.float32
NPBF16 = ml_dtypes.bfloat16

QLOC = 2048          # query rows per core
NKT = S // P         # 32 k tiles
DCH = D // P         # 8 contraction chunks
SCALE = 1.0 / float(np.sqrt(H))     # pre-exp scale
LIMITS = [8, 16, 24, 32]            # k-tile limit per slot (by slot id r)

# qT local column layout: natural [slot0 | slot1 | slot2 | slot3]
QOFF = {0: 0, 1: 512, 2: 1024, 3: 1536}
PASS_A = (0, 1)
PASS_B = (2, 3)


def qglob_for_core(h):
    """Global query row indices (length QLOC) in local qT order."""
    idx = []
    for r in (0, 1, 2, 3):
        for s in range(4):
            g = 8 * r + 2 * s + h
            idx.append(np.arange(g * P, (g + 1) * P))
    return np.concatenate(idx)


def c0_of(r, kt):
    """First needed column of slot r's 512-block at k-brick kt (pair-granular,
    identical for both halves)."""
    return P * max(0, (kt - 8 * r) // 2)


def build_nc():
    nc = bacc.Bacc(None, target_bir_lowering=False, debug=False, num_devices=8)

    xt = nc.dram_tensor("xt", [D, S], BF16, kind="ExternalInput").ap()
    w_ap = {}
    for nm in ("wq", "wk", "wv"):
        # host pre-shuffles to [P, DCH*H] so the load is 2KB contiguous lines
        w_ap[nm] = nc.dram_tensor(nm, [P, DCH * H], BF16, kind="ExternalInput").ap()
    bq = nc.dram_tensor("bq", [H, 1], F32, kind="ExternalInput").ap()
    kio = nc.dram_tensor("kio", [P, NKT], mybir.dt.int16, kind="ExternalInput").ap()
    identb = nc.dram_tensor("identb", [P, P], BF16, kind="ExternalInput").ap()
    onesb = nc.dram_tensor("onesb", [P, 1], BF16, kind="ExternalInput").ap()
    out = nc.dram_tensor("out", [QLOC, H], BF16, kind="ExternalOutput").ap()

    Ident = mybir.ActivationFunctionType.Identity
    Copy = mybir.ActivationFunctionType.Copy
    Exp = mybir.ActivationFunctionType.Exp

    with tile.TileContext(nc) as tc, ExitStack() as ctx:
        consts = ctx.enter_context(tc.tile_pool(name="consts", bufs=1))
        persist = ctx.enter_context(tc.tile_pool(name="persist", bufs=1))

        # ---- persistent activations
        kT = persist.tile([P, S], BF16, tag="kT")          # K^T [h, s_local]
        vN = persist.tile([P, NKT, H], BF16, tag="vN")     # V natural [k_l, kt, h]
        qT = persist.tile([P, QLOC], BF16, tag="qT")       # Q^T [h, q_local]
        xt_sb = persist.tile([P, DCH, S], BF16, tag="xt_sb")    # x^T resident
        # gather view: S = (pair b:4)(s:4)(parity:2)(w:128)
        xt_g = xt_sb.rearrange("p c (b s two w) -> p c b s two w", b=4, s=4, two=2)

        # ---- loads: wk + stripe0/1 first (gate the first projections), the
        # rest of x^T as 2MB blocks (2KB lines, one descriptor set each)
        xt_r = xt.rearrange("(c p) s -> p c s", p=P)

        def load_w(nm, eng):
            t = consts.tile([P, DCH, H], BF16, tag=f"w_{nm}", name=f"w_{nm}")
            eng.dma_start(out=t[:], in_=w_ap[nm].rearrange("p (c h) -> p c h", c=DCH))
            return t

        def load_x_cols(c0, c1):
            nc.sync.dma_start(out=xt_sb[:, :, c0:c1], in_=xt_r[:, :, c0:c1])

        # parallel descriptor generation across the three DMA-capable queues:
        # sync carries wk + x stripes, scalar carries wv/wq + stripe 1,
        # gpsimd carries the small consts
        w_sb = {}
        w_sb["wk"] = load_w("wk", nc.sync)
        nc.sync.dma_start(out=xt_sb[:, 0:4, 0:512], in_=xt_r[:, 0:4, 0:512])
        nc.sync.dma_start(out=xt_sb[:, 4:8, 0:512], in_=xt_r[:, 4:8, 0:512])
        w_sb["wv"] = load_w("wv", nc.scalar)
        load_x_cols(512, 1024)
        w_sb["wq"] = load_w("wq", nc.scalar)
        load_x_cols(1024, 2048)
        load_x_cols(2048, 3072)
        load_x_cols(3072, 4096)

        bq_sb = consts.tile([P, 1], F32, tag="bq")
        nc.gpsimd.dma_start(out=bq_sb[:], in_=bq)
        identb_sb = consts.tile([P, P], BF16, tag="identb")
        nc.gpsimd.dma_start(out=identb_sb[:], in_=identb)
        ones_sb = consts.tile([P, 1], BF16, tag="ones")
        nc.gpsimd.dma_start(out=ones_sb[:], in_=onesb)
        kio_sb = consts.tile([P, NKT], mybir.dt.int16, tag="kio")
        nc.gpsimd.dma_start(out=kio_sb[:], in_=kio)

        # qpos generated on-chip in h-shifted coordinates (global - 128h):
        # affine per 512-block (base 1024*slot + 256*s + w); the host shifts
        # kio by -128h to match, so is_ge(qpos, kio) is exact causal.
        qpos_b = consts.tile([P, QLOC], mybir.dt.int16, tag="qpos_b")
        for blk, r in enumerate((0, 1, 2, 3)):
            nc.gpsimd.iota(qpos_b[:, blk * 512:(blk + 1) * 512],
                           pattern=[[256, 4], [1, P]], base=1024 * r,
                           channel_multiplier=0)

        with tc.tile_pool(name="ps", bufs=1, space="PSUM") as ps, \
             tc.tile_pool(name="stg", bufs=3) as stg, \
             tc.tile_pool(name="pp", bufs=8) as pp, \
             tc.tile_pool(name="acc", bufs=1) as accp, \
             tc.tile_pool(name="epi", bufs=3) as epi:

            # PSUM budget (8 banks): mm512 x2 + sT2 (2 banks) x2 + oT a/b = 8
            def psA():
                return ps.tile([P, 512], F32, tag="mm512", name="mm512", bufs=2)

            def psA_b16():
                return ps.tile([P, 512], BF16, tag="mm512", name="mm512b", bufs=2)

            def psS2():
                return ps.tile([P, 2, 512], F32, tag="sT2", name="sT2", bufs=2)

            oT = {}   # slot -> PSUM tile [P, 512] f32, 2 live at a time

            # ---------------- projections ----------------
            def proj_stripe_mm(wname, sr, ps_t):
                for j in range(DCH):
                    nc.tensor.matmul(
                        ps_t[:], lhsT=w_sb[wname][:, j, :],
                        rhs=xt_sb[:, j, sr * 512:(sr + 1) * 512],
                        start=(j == 0), stop=(j == DCH - 1),
                    )
                    if j == 3:
                        yield None
                yield None

            def copy_fn(on_act):
                def copy(dst, src, bias=None):
                    if on_act:
                        nc.scalar.activation(dst, src, Ident if bias is not None
                                             else Copy,
                                             **({"bias": bias, "scale": 1.0}
                                                if bias is not None else {}))
                    elif bias is not None:
                        nc.vector.tensor_scalar_add(dst, src, bias)
                    else:
                        nc.vector.tensor_copy(dst, src)
                return copy

            def kq_steps(b, on_act=True):
                """K projections of stripes 2b,2b+1 + Q of slot b (the parts
                a pass is gated on).  on_act: copies on ACT (idle in P-phases);
                else DVE (for filler inside a pass where ACT runs exps)."""
                copy = copy_fn(on_act)
                for sr in (2 * b, 2 * b + 1):
                    pk = psA()
                    yield from proj_stripe_mm("wk", sr, pk)
                    copy(kT[:, sr * 512:(sr + 1) * 512], pk[:])
                # Q projection for slot r=b (gather even local blocks of the pair)
                pq = psA()
                for j in range(DCH):
                    nc.tensor.matmul(
                        pq[:], lhsT=w_sb["wq"][:, j, :],
                        rhs=xt_g[:, j, b, :, 0, :],
                        start=(j == 0), stop=(j == DCH - 1),
                    )
                    if j == 3:
                        yield None
                q0 = QOFF[b]
                copy(qT[:, q0:q0 + 512], pq[:], bias=bq_sb[:])
                yield None

            def v_steps(b, on_act=True):
                """V projections of stripes 2b,2b+1 + PE transposes to vN."""
                copy = copy_fn(on_act)
                vTs = stg.tile([P, 1024], BF16, tag="vTs", name="vTs")
                for i, sr in enumerate((2 * b, 2 * b + 1)):
                    pv = psA()
                    yield from proj_stripe_mm("wv", sr, pv)
                    copy(vTs[:, i * 512:(i + 1) * 512], pv[:])
                for half in range(2):
                    pst = psA_b16()
                    for t_ in range(4):
                        nc.tensor.matmul(
                            pst[:, t_ * P:(t_ + 1) * P],
                            lhsT=vTs[:, half * 512 + t_ * P: half * 512 + (t_ + 1) * P],
                            rhs=identb_sb[:], is_transpose=True, skip_group_check=True,
                        )
                    nc.vector.tensor_copy(
                        vN[:, (2 * b + half) * 4:(2 * b + half + 1) * 4, :], pst[:]
                    )
                    yield None

            def phase_steps(b, on_act=True):
                """Full phase, stripe-major interleaved (K s, V s alternate) so
                a pending second-stripe DMA hides under first-stripe V work."""
                copy = copy_fn(on_act)
                vTs = stg.tile([P, 1024], BF16, tag="vTs", name="vTs")
                for i, sr in enumerate((2 * b, 2 * b + 1)):
                    pk = psA()
                    yield from proj_stripe_mm("wk", sr, pk)
                    copy(kT[:, sr * 512:(sr + 1) * 512], pk[:])
                    pv = psA()
                    yield from proj_stripe_mm("wv", sr, pv)
                    copy(vTs[:, i * 512:(i + 1) * 512], pv[:])
                pq = psA()
                for j in range(DCH):
                    nc.tensor.matmul(
                        pq[:], lhsT=w_sb["wq"][:, j, :],
                        rhs=xt_g[:, j, b, :, 0, :],
                        start=(j == 0), stop=(j == DCH - 1),
                    )
                    if j == 3:
                        yield None
                q0 = QOFF[b]
                copy(qT[:, q0:q0 + 512], pq[:], bias=bq_sb[:])
                yield None
                for half in range(2):
                    pst = psA_b16()
                    for t_ in range(4):
                        nc.tensor.matmul(
                            pst[:, t_ * P:(t_ + 1) * P],
                            lhsT=vTs[:, half * 512 + t_ * P: half * 512 + (t_ + 1) * P],
                            rhs=identb_sb[:], is_transpose=True, skip_group_check=True,
                        )
                    nc.vector.tensor_copy(
                        vN[:, (2 * b + half) * 4:(2 * b + half + 1) * 4, :], pst[:]
                    )
                    yield None

            # ---------------- epilogue ----------------
            def epilogue_steps(r, dacc, di, dacc2=False):
                """Generator: output slot r.  dacc [P,1024] bf16, di = 0/1 half."""
                oTs = epi.tile([P, 512], BF16, tag="oTs", name="oTs")
                nc.vector.tensor_copy(oTs[:], oT[r][:])
                dT = psA()
                for s_ in range(4):
                    srcs = [dacc[:, di * 512 + s_ * P: di * 512 + (s_ + 1) * P]]
                    if dacc2:
                        srcs.append(dacc[:, 1024 + s_ * P: 1024 + (s_ + 1) * P])
                    for k_, sc in enumerate(srcs):
                        nc.tensor.matmul(
                            dT[:, s_:s_ + 1], lhsT=sc, rhs=ones_sb[:],
                            start=(k_ == 0), stop=(k_ == len(srcs) - 1),
                            skip_group_check=True,
                        )
                yield None
                rec = epi.tile([P, 4], F32, tag="rec", name="rec")
                nc.vector.reciprocal(rec[:], dT[:, 0:4])
                yield None
                obr = psA_b16()
                for s_ in range(4):
                    nc.tensor.matmul(
                        obr[:, s_ * P:(s_ + 1) * P], lhsT=oTs[:, s_ * P:(s_ + 1) * P],
                        rhs=identb_sb[:], is_transpose=True, skip_group_check=True,
                    )
                yield None
                ofin = epi.tile([P, 512], BF16, tag="ofin", name="ofin")
                rec_b = bass.AP(tensor=rec.tensor, offset=rec.offset,
                                ap=[rec.ap[0], [1, 4], [0, P]])
                nc.vector.scalar_tensor_tensor(
                    ofin[:].rearrange("p (s w) -> p s w", s=4),
                    obr[:].rearrange("p (s w) -> p s w", s=4),
                    SCALE, rec_b,
                    op0=mybir.AluOpType.mult, op1=mybir.AluOpType.mult)
                yield None
                q0 = QOFF[r]
                nc.sync.dma_start(
                    out=out[q0:q0 + 512, :].rearrange("(s p) h -> p s h", p=P),
                    in_=ofin[:].rearrange("p (s h) -> p s h", s=4),
                )
                yield None

            # ---------------- attention pass ----------------
            def attention_pass(slots, dacc, filler, rate=1):
                """kt-outer pass over slot pair (lo, hi); lo has smaller limit.
                qT cols: lo at QOFF[lo], hi at QOFF[hi] (adjacent, lo first).
                filler: iterator of generators for PE filler work."""
                lo, hi = slots
                Llo, Lhi = LIMITS[lo], LIMITS[hi]
                q0 = QOFF[lo]
                assert QOFF[hi] == q0 + 512
                oT[lo] = ps.tile([P, 512], F32, tag=f"oT{len(oT) % 2}",
                                 name=f"oT_{lo}", bufs=1)
                oT[hi] = ps.tile([P, 512], F32, tag=f"oT{(len(oT)) % 2}",
                                 name=f"oT_{hi}", bufs=1)

                def pull():
                    try:
                        next(filler)
                    except StopIteration:
                        pass

                # iteration schedule: two-slot region one kt at a time;
                # single-slot region (kt >= Llo) in fused kt pairs
                iters = [(kt, kt) for kt in range(Llo)]
                iters += [(kt, kt + 1) for kt in range(Llo, Lhi, 2)]

                def score(it):
                    kt0, kt1 = it
                    sT2 = psS2()
                    pT = pp.tile([P, 2, 512], BF16, tag="pT", name="pT")
                    if kt0 == kt1:          # both slots at kt0
                        kt = kt0
                        cl = c0_of(lo, kt)
                        ch = c0_of(hi, kt)
                        assert ch == 0
                        nc.tensor.matmul(
                            sT2[:, 0, cl:], lhsT=kT[:, kt * P:(kt + 1) * P],
                            rhs=qT[:, q0 + cl:q0 + 512], start=True, stop=True,
                        )
                        nc.tensor.matmul(
                            sT2[:, 1, :], lhsT=kT[:, kt * P:(kt + 1) * P],
                            rhs=qT[:, q0 + 512:q0 + 1024], start=True, stop=True,
                        )
                        flat = sT2.rearrange("p a w -> p (a w)")
                        pf = pT.rearrange("p a w -> p (a w)")
                        nc.scalar.activation(pf[:, cl:], flat[:, cl:], Exp, scale=SCALE)
                        # mask windows are disjoint: lo in [0,Llo), hi in [Lhi-8,Lhi)
                        if kt >= Llo - 8:
                            # only the subtile at cl straddles the diagonal
                            nc.vector.scalar_tensor_tensor(
                                pT[:, 0, cl:cl + P], qpos_b[:, q0 + cl:q0 + cl + P],
                                kio_sb[:, kt:kt + 1], pT[:, 0, cl:cl + P],
                                op0=mybir.AluOpType.is_ge, op1=mybir.AluOpType.mult,
                            )
                        if kt == 0:
                            nc.vector.tensor_copy(dacc[:, 0:1024], pf[:])
                        else:
                            nc.vector.tensor_add(dacc[:, cl:1024], dacc[:, cl:1024],
                                                 pf[:, cl:])
                        return (pT, kt0, kt1, cl)
                    else:                   # hi only, fused kt pair
                        c0 = c0_of(hi, kt0)
                        c1 = c0_of(hi, kt1)
                        assert c0 == c1
                        for i, kt in enumerate((kt0, kt1)):
                            nc.tensor.matmul(
                                sT2[:, i, c0:], lhsT=kT[:, kt * P:(kt + 1) * P],
                                rhs=qT[:, q0 + 512 + c0:q0 + 1024],
                                start=True, stop=True,
                            )
                        nc.scalar.activation(pT[:, :, c0:], sT2[:, :, c0:], Exp,
                                             scale=SCALE)
                        if kt1 >= Lhi - 8:
                            for i, kt in enumerate((kt0, kt1)):
                                nc.vector.scalar_tensor_tensor(
                                    pT[:, i, c0:c0 + P],
                                    qpos_b[:, q0 + 512 + c0:q0 + 512 + c0 + P],
                                    kio_sb[:, kt:kt + 1], pT[:, i, c0:c0 + P],
                                    op0=mybir.AluOpType.is_ge, op1=mybir.AluOpType.mult,
                                )
                        nc.vector.tensor_add(
                            dacc[:, 512 + c0:1024], dacc[:, 512 + c0:1024],
                            pT[:, 0, c0:])
                        if kt0 == Llo:
                            nc.vector.tensor_copy(dacc[:, 1024:1536], pT[:, 1, :])
                        else:
                            nc.vector.tensor_add(
                                dacc[:, 1024 + c0:1536], dacc[:, 1024 + c0:1536],
                                pT[:, 1, c0:])
                        return (pT, kt0, kt1, None)

                def accum(st):
                    pT, kt0, kt1, cl = st
                    if kt0 == kt1:
                        kt = kt0
                        if kt < Llo:
                            nc.tensor.matmul(
                                oT[lo][:, cl:], lhsT=vN[:, kt, :], rhs=pT[:, 0, cl:],
                                start=(kt == 0), stop=(kt == Llo - 1),
                            )
                        nc.tensor.matmul(
                            oT[hi][:], lhsT=vN[:, kt, :], rhs=pT[:, 1, :],
                            start=(kt == 0), stop=(kt == Lhi - 1),
                        )
                    else:
                        c0 = c0_of(hi, kt0)
                        for i, kt in enumerate((kt0, kt1)):
                            nc.tensor.matmul(
                                oT[hi][:, c0:], lhsT=vN[:, kt, :], rhs=pT[:, i, c0:],
                                start=(kt == 0), stop=(kt == Lhi - 1),
                            )

                window = []
                for n, it in enumerate(iters):
                    window.append(score(it))
                    if n >= 4:
                        for _ in range(rate):
                            pull()
                    if len(window) > 3:
                        accum(window.pop(0))
                        if n >= 4:
                            for _ in range(rate):
                                pull()
                while window:
                    accum(window.pop(0))
                    pull()
                for _ in range(6):
                    pull()

            # ---------------- emission ----------------
            def chain(*gens):
                for g in gens:
                    yield from g

            def drain(g):
                for _ in g:
                    pass

            # pre-warm the ACT exp table during P-phase 0
            warm = epi.tile([P, 1], F32, tag="warm", name="warm")
            nc.scalar.activation(warm[:], ones_sb[:], Exp)

            # P0 in full (pass A needs vN bricks 0..7 early), but only the
            # K/Q parts of P1 gate pass A: its V rides as filler
            drain(phase_steps(0))
            drain(kq_steps(1))

            dacc_a = accp.tile([P, 1536], BF16, tag="dacc_a")
            dacc_b = accp.tile([P, 1536], BF16, tag="dacc_b")

            # P1's V plus P2/P3 ride as PE filler under pass A, with their
            # copies on DVE so the exp-critical ACT queue stays clear
            attention_pass(PASS_A, dacc_a,
                           chain(v_steps(1, False), phase_steps(2, False),
                                 phase_steps(3, False)), rate=3)

            # emit d/reciprocal/O-copy of slots 0,1 BEFORE pass B: pass B's
            # first accumulations WAR-wait on the oT ring slots these release,
            # so they must be ahead of pass B's DVE-queue ops (deadlock
            # avoidance), and they free the PSUM oT banks early.
            e0 = epilogue_steps(0, dacc_a, 0)
            e1 = epilogue_steps(1, dacc_a, 1, True)
            next(e0), next(e0)
            next(e1), next(e1)

            def skip(k):
                for _ in range(k):
                    yield None

            # slot 2's last accumulation pops at emission iter 25; spacers
            # keep its epilogue from being emitted before that
            attention_pass(PASS_B, dacc_b,
                           chain(e0, e1, skip(44), epilogue_steps(2, dacc_b, 0)))
            drain(epilogue_steps(3, dacc_b, 1, True))

    nc.compile()
    return nc


_NC_CACHE = None


def _get_nc():
    global _NC_CACHE
    if _NC_CACHE is None:
        _NC_CACHE = build_nc()
    return _NC_CACHE


def make_in_maps(inputs):
    x = np.asarray(inputs["x"], np.float32)
    Wq = np.asarray(inputs["Wq"], np.float32)
    Wk = np.asarray(inputs["Wk"], np.float32)
    Wv = np.asarray(inputs["Wv"], np.float32)
    bq = np.asarray(inputs["bq"], np.float32)

    xb = x.astype(NPBF16)

    def wshuf(W):
        # [D,H] -> [P, DCH*H]: partition p holds chunks c at rows c*128+p
        return np.ascontiguousarray(
            W.astype(NPBF16).reshape(DCH, P, H).transpose(1, 0, 2).reshape(P, DCH * H))

    common = dict(
        wq=wshuf(Wq), wk=wshuf(Wk), wv=wshuf(Wv),
        bq=bq.reshape(H, 1),
        identb=np.eye(P, dtype=NPBF16),
        onesb=np.ones((P, 1), dtype=NPBF16),
    )
    in_maps = []
    xbT = np.ascontiguousarray(xb.transpose(0, 2, 1))  # [B, D, S]
    swap = np.arange(NKT).reshape(-1, 2)[:, ::-1].reshape(-1)  # pair-swap blocks
    for c in range(8):
        b, hh = c // 2, c % 2
        m = dict(common)
        if hh == 0:
            m["xt"] = xbT[b]
            blk = np.arange(NKT)
        else:
            m["xt"] = np.ascontiguousarray(
                xbT[b].reshape(D, NKT, P)[:, swap, :].reshape(D, S))
            blk = swap
        # kio: global positions of local k-brick kt
        m["kio"] = (blk[None, :] * P + np.arange(P)[:, None] - P * hh).astype(np.int16)
        in_maps.append(m)
    return in_maps


def assemble_out(results):
    out = np.zeros((1, B, S, H), np.float32)
    for c in range(8):
        b, hh = c // 2, c % 2
        qg = qglob_for_core(hh)
        out[0, b, qg, :] = results[c]["out"].astype(np.float32)
    return out


def kernel(**inputs) -> np.ndarray:
    nc = _get_nc()
    in_maps = make_in_maps(inputs)
    res = run_bass_kernel_spmd(nc, in_maps, list(range(8)))
    out = assemble_out(res.results)
    bv = np.asarray(inputs["bv"], np.float32)
    out += (bv * SCALE)[None, None, None, :]
    return out


# revision 41
# speedup vs baseline: 1.0089x; 1.0089x over previous
"""Causal single-head attention (B=4, S=4096, D=1024, H=128) on 8 NeuronCores.

Sharding: core c = (batch b = c//2, half h = c%2). One shared SPMD program;
all per-half differences are carried in the DATA:
  - host pair-swaps adjacent 128-col blocks of x^T for h=1 cores, so a fixed
    even-local-block gather pattern selects that half's parity-interleaved
    q subtiles (global subtile g = 8r+2s+h for slot r, s in 0..3),
  - kio holds h-shifted global k positions; qpos is generated on-chip by
    iota, so is_ge(qpos, kio) is the exact causal mask.

Each core: K^T/V (full 4096 keys, replicated within the batch pair) + its own
2048 query rows.  Algebraic prunes: bk dropped (softmax shift-invariance along
k), bv folded into a host-side constant add (sum_k att = 1/sqrt(H)).

Pipeline per core (bf16 matmuls, fp32 PSUM):
  x^T host-transposed bf16, loaded as a few block DMAs with 1-2KB lines;
  weights host-preshuffled to [128, 8*128] for 2KB-line loads
  P-phases b=0..3: K/V/Q projections of stripe pair (2b,2b+1), stripe-major;
  V natural via PE transpose; Q gathered straight from resident x^T
  (strided AP); PSUM->SBUF copies ride the ACT engine
  attention in two passes, kt-outer, software-pipelined 2 deep; per kt ONE
  fused exp over both slots' adjacent PSUM banks ([P,2,512] tile);
  single-slot stretches fuse kt PAIRS into one exp.
    pass A = slots (0,1), kts 0..15, P-phases 2+3 interleaved as PE filler
    pass B = slots (2,3), kts 0..31, epilogues of slots 0,1,2 interleaved
  causal mask: one 128-col DVE is_ge STT per kt (only the subtile at the
  c0 boundary straddles the diagonal); denominator in bf16 on DVE with an
  extra accumulator per pass for the fused-pair second kt
  epilogue per slot: d^T by one-col matmuls (dacc as weights), reciprocal,
  O^T -> O by PE transpose, scale by rec*(1/sqrt(H)) via STT, bf16 DMA out
"""

import numpy as np
import ml_dtypes
from contextlib import ExitStack

import concourse.bass as bass
import concourse.tile as tile
from concourse import bacc, mybir
from concourse.bass_utils import run_bass_kernel_spmd

B, S, D, H = 4, 4096, 1024, 128
P = 128
BF16 = mybir.dt.bfloat16
F32 = mybir.dt.float32
NPBF16 = ml_dtypes.bfloat16

QLOC = 2048          # query rows per core
NKT = S // P         # 32 k tiles
DCH = D // P         # 8 contraction chunks
SCALE = 1.0 / float(np.sqrt(H))     # pre-exp scale
LIMITS = [8, 16, 24, 32]            # k-tile limit per slot (by slot id r)

# qT local column layout: natural [slot0 | slot1 | slot2 | slot3]
QOFF = {0: 0, 1: 512, 2: 1024, 3: 1536}
PASS_A = (0, 1)
PASS_B = (2, 3)


def qglob_for_core(h):
    """Global query row indices (length QLOC) in local qT order."""
    idx = []
    for r in (0, 1, 2, 3):
        for s in range(4):
            g = 8 * r + 2 * s + h
            idx.append(np.arange(g * P, (g + 1) * P))
    return np.concatenate(idx)


def c0_of(r, kt):
    """First needed column of slot r's 512-block at k-brick kt (pair-granular,
    identical for both halves)."""
    return P * max(0, (kt - 8 * r) // 2)


def build_nc():
    nc = bacc.Bacc(None, target_bir_lowering=False, debug=False, num_devices=8)

    xt = nc.dram_tensor("xt", [D, S], BF16, kind="ExternalInput").ap()
    w_ap = {}
    for nm in ("wq", "wk", "wv"):
        # host pre-shuffles to [P, DCH*H] so the load is 2KB contiguous lines
        w_ap[nm] = nc.dram_tensor(nm, [P, DCH * H], BF16, kind="ExternalInput").ap()
    bq = nc.dram_tensor("bq", [H, 1], F32, kind="ExternalInput").ap()
    kio = nc.dram_tensor("kio", [P, NKT], mybir.dt.int16, kind="ExternalInput").ap()
    identb = nc.dram_tensor("identb", [P, P], BF16, kind="ExternalInput").ap()
    onesb = nc.dram_tensor("onesb", [P, 1], BF16, kind="ExternalInput").ap()
    out = nc.dram_tensor("out", [QLOC, H], BF16, kind="ExternalOutput").ap()

    Ident = mybir.ActivationFunctionType.Identity
    Copy = mybir.ActivationFunctionType.Copy
    Exp = mybir.ActivationFunctionType.Exp

    with tile.TileContext(nc) as tc, ExitStack() as ctx:
        consts = ctx.enter_context(tc.tile_pool(name="consts", bufs=1))
        persist = ctx.enter_context(tc.tile_pool(name="persist", bufs=1))

        # ---- persistent activations
        kT = persist.tile([P, S], BF16, tag="kT")          # K^T [h, s_local]
        vN = persist.tile([P, NKT, H], BF16, tag="vN")     # V natural [k_l, kt, h]
        qT = persist.tile([P, QLOC], BF16, tag="qT")       # Q^T [h, q_local]
        xt_sb = persist.tile([P, DCH, S], BF16, tag="xt_sb")    # x^T resident
        # gather view: S = (pair b:4)(s:4)(parity:2)(w:128)
        xt_g = xt_sb.rearrange("p c (b s two w) -> p c b s two w", b=4, s=4, two=2)

        # ---- loads: wk + stripe0/1 first (gate the first projections), the
        # rest of x^T as 2MB blocks (2KB lines, one descriptor set each)
        xt_r = xt.rearrange("(c p) s -> p c s", p=P)

        def load_w(nm, eng):
            t = consts.tile([P, DCH, H], BF16, tag=f"w_{nm}", name=f"w_{nm}")
            eng.dma_start(out=t[:], in_=w_ap[nm].rearrange("p (c h) -> p c h", c=DCH))
            return t

        def load_x_cols(c0, c1):
            nc.sync.dma_start(out=xt_sb[:, :, c0:c1], in_=xt_r[:, :, c0:c1])

        # parallel descriptor generation across the three DMA-capable queues:
        # sync carries wk + x stripes, scalar carries wv/wq + stripe 1,
        # gpsimd carries the small consts
        w_sb = {}
        w_sb["wk"] = load_w("wk", nc.sync)
        nc.sync.dma_start(out=xt_sb[:, 0:4, 0:512], in_=xt_r[:, 0:4, 0:512])
        nc.sync.dma_start(out=xt_sb[:, 4:8, 0:512], in_=xt_r[:, 4:8, 0:512])
        w_sb["wv"] = load_w("wv", nc.scalar)
        load_x_cols(512, 1024)
        w_sb["wq"] = load_w("wq", nc.scalar)
        load_x_cols(1024, 2048)
        load_x_cols(2048, 3072)
        load_x_cols(3072, 4096)

        bq_sb = consts.tile([P, 1], F32, tag="bq")
        nc.gpsimd.dma_start(out=bq_sb[:], in_=bq)
        identb_sb = consts.tile([P, P], BF16, tag="identb")
        nc.gpsimd.dma_start(out=identb_sb[:], in_=identb)
        ones_sb = consts.tile([P, 1], BF16, tag="ones")
        nc.gpsimd.dma_start(out=ones_sb[:], in_=onesb)
        kio_sb = consts.tile([P, NKT], mybir.dt.int16, tag="kio")
        nc.gpsimd.dma_start(out=kio_sb[:], in_=kio)

        # qpos generated on-chip in h-shifted coordinates (global - 128h):
        # affine per 512-block (base 1024*slot + 256*s + w); the host shifts
        # kio by -128h to match, so is_ge(qpos, kio) is exact causal.
        qpos_b = consts.tile([P, QLOC], mybir.dt.int16, tag="qpos_b")
        for blk, r in enumerate((0, 1, 2, 3)):
            nc.gpsimd.iota(qpos_b[:, blk * 512:(blk + 1) * 512],
                           pattern=[[256, 4], [1, P]], base=1024 * r,
                           channel_multiplier=0)

        with tc.tile_pool(name="ps", bufs=1, space="PSUM") as ps, \
             tc.tile_pool(name="stg", bufs=3) as stg, \
             tc.tile_pool(name="pp", bufs=8) as pp, \
             tc.tile_pool(name="acc", bufs=1) as accp, \
             tc.tile_pool(name="epi", bufs=3) as epi:

            # PSUM budget (8 banks): mm512 x2 + sT2 (2 banks) x2 + oT a/b = 8
            def psA():
                return ps.tile([P, 512], F32, tag="mm512", name="mm512", bufs=2)

            def psA_b16():
                return ps.tile([P, 512], BF16, tag="mm512", name="mm512b", bufs=2)

            def psS2():
                return ps.tile([P, 2, 512], F32, tag="sT2", name="sT2", bufs=2)

            oT = {}   # slot -> PSUM tile [P, 512] f32, 2 live at a time

            # ---------------- projections ----------------
            def proj_stripe_mm(wname, sr, ps_t):
                for j in range(DCH):
                    nc.tensor.matmul(
                        ps_t[:], lhsT=w_sb[wname][:, j, :],
                        rhs=xt_sb[:, j, sr * 512:(sr + 1) * 512],
                        start=(j == 0), stop=(j == DCH - 1),
                    )
                    if j == 3:
                        yield None
                yield None

            def copy_fn(on_act):
                def copy(dst, src, bias=None):
                    if on_act:
                        nc.scalar.activation(dst, src, Ident if bias is not None
                                             else Copy,
                                             **({"bias": bias, "scale": 1.0}
                                                if bias is not None else {}))
                    elif bias is not None:
                        nc.vector.tensor_scalar_add(dst, src, bias)
                    else:
                        nc.vector.tensor_copy(dst, src)
                return copy

            def kq_steps(b, on_act=True):
                """K projections of stripes 2b,2b+1 + Q of slot b (the parts
                a pass is gated on).  on_act: copies on ACT (idle in P-phases);
                else DVE (for filler inside a pass where ACT runs exps)."""
                copy = copy_fn(on_act)
                for sr in (2 * b, 2 * b + 1):
                    pk = psA()
                    yield from proj_stripe_mm("wk", sr, pk)
                    copy(kT[:, sr * 512:(sr + 1) * 512], pk[:])
                # Q projection for slot r=b (gather even local blocks of the pair)
                pq = psA()
                for j in range(DCH):
                    nc.tensor.matmul(
                        pq[:], lhsT=w_sb["wq"][:, j, :],
                        rhs=xt_g[:, j, b, :, 0, :],
                        start=(j == 0), stop=(j == DCH - 1),
                    )
                    if j == 3:
                        yield None
                q0 = QOFF[b]
                copy(qT[:, q0:q0 + 512], pq[:], bias=bq_sb[:])
                yield None

            def v_steps(b, on_act=True):
                """V projections of stripes 2b,2b+1 + PE transposes to vN."""
                copy = copy_fn(on_act)
                vTs = stg.tile([P, 1024], BF16, tag="vTs", name="vTs")
                for i, sr in enumerate((2 * b, 2 * b + 1)):
                    pv = psA()
                    yield from proj_stripe_mm("wv", sr, pv)
                    copy(vTs[:, i * 512:(i + 1) * 512], pv[:])
                for half in range(2):
                    pst = psA_b16()
                    for t_ in range(4):
                        nc.tensor.matmul(
                            pst[:, t_ * P:(t_ + 1) * P],
                            lhsT=vTs[:, half * 512 + t_ * P: half * 512 + (t_ + 1) * P],
                            rhs=identb_sb[:], is_transpose=True, skip_group_check=True,
                        )
                    nc.vector.tensor_copy(
                        vN[:, (2 * b + half) * 4:(2 * b + half + 1) * 4, :], pst[:]
                    )
                    yield None

            def phase_steps(b, on_act=True):
                """Full phase, stripe-major interleaved (K s, V s alternate) so
                a pending second-stripe DMA hides under first-stripe V work."""
                copy = copy_fn(on_act)
                vTs = stg.tile([P, 1024], BF16, tag="vTs", name="vTs")
                for i, sr in enumerate((2 * b, 2 * b + 1)):
                    pk = psA()
                    yield from proj_stripe_mm("wk", sr, pk)
                    copy(kT[:, sr * 512:(sr + 1) * 512], pk[:])
                    pv = psA()
                    yield from proj_stripe_mm("wv", sr, pv)
                    copy(vTs[:, i * 512:(i + 1) * 512], pv[:])
                pq = psA()
                for j in range(DCH):
                    nc.tensor.matmul(
                        pq[:], lhsT=w_sb["wq"][:, j, :],
                        rhs=xt_g[:, j, b, :, 0, :],
                        start=(j == 0), stop=(j == DCH - 1),
                    )
                    if j == 3:
                        yield None
                q0 = QOFF[b]
                copy(qT[:, q0:q0 + 512], pq[:], bias=bq_sb[:])
                yield None
                for half in range(2):
                    pst = psA_b16()
                    for t_ in range(4):
                        nc.tensor.matmul(
                            pst[:, t_ * P:(t_ + 1) * P],
                            lhsT=vTs[:, half * 512 + t_ * P: half * 512 + (t_ + 1) * P],
                            rhs=identb_sb[:], is_transpose=True, skip_group_check=True,
                        )
                    nc.vector.tensor_copy(
                        vN[:, (2 * b + half) * 4:(2 * b + half + 1) * 4, :], pst[:]
                    )
                    yield None

            # ---------------- epilogue ----------------
            def epilogue_steps(r, dacc, di, dacc2=False):
                """Generator: output slot r.  dacc [P,1024] bf16, di = 0/1 half."""
                oTs = epi.tile([P, 512], BF16, tag="oTs", name="oTs")
                nc.vector.tensor_copy(oTs[:], oT[r][:])
                dT = psA()
                for s_ in range(4):
                    srcs = [dacc[:, di * 512 + s_ * P: di * 512 + (s_ + 1) * P]]
                    if dacc2:
                        srcs.append(dacc[:, 1024 + s_ * P: 1024 + (s_ + 1) * P])
                    for k_, sc in enumerate(srcs):
                        nc.tensor.matmul(
                            dT[:, s_:s_ + 1], lhsT=sc, rhs=ones_sb[:],
                            start=(k_ == 0), stop=(k_ == len(srcs) - 1),
                            skip_group_check=True,
                        )
                yield None
                rec = epi.tile([P, 4], F32, tag="rec", name="rec")
                nc.vector.reciprocal(rec[:], dT[:, 0:4])
                yield None
                obr = psA_b16()
                for s_ in range(4):
                    nc.tensor.matmul(
                        obr[:, s_ * P:(s_ + 1) * P], lhsT=oTs[:, s_ * P:(s_ + 1) * P],
                        rhs=identb_sb[:], is_transpose=True, skip_group_check=True,
                    )
                yield None
                ofin = epi.tile([P, 512], BF16, tag="ofin", name="ofin")
                rec_b = bass.AP(tensor=rec.tensor, offset=rec.offset,
                                ap=[rec.ap[0], [1, 4], [0, P]])
                nc.vector.scalar_tensor_tensor(
                    ofin[:].rearrange("p (s w) -> p s w", s=4),
                    obr[:].rearrange("p (s w) -> p s w", s=4),
                    SCALE, rec_b,
                    op0=mybir.AluOpType.mult, op1=mybir.AluOpType.mult)
                yield None
                q0 = QOFF[r]
                nc.sync.dma_start(
                    out=out[q0:q0 + 512, :].rearrange("(s p) h -> p s h", p=P),
                    in_=ofin[:].rearrange("p (s h) -> p s h", s=4),
                )
                yield None

            # ---------------- attention pass ----------------
            def attention_pass(slots, dacc, filler, rate=1):
                """kt-outer pass over slot pair (lo, hi); lo has smaller limit.
                qT cols: lo at QOFF[lo], hi at QOFF[hi] (adjacent, lo first).
                filler: iterator of generators for PE filler work."""
                lo, hi = slots
                Llo, Lhi = LIMITS[lo], LIMITS[hi]
                q0 = QOFF[lo]
                assert QOFF[hi] == q0 + 512
                oT[lo] = ps.tile([P, 512], F32, tag=f"oT{len(oT) % 2}",
                                 name=f"oT_{lo}", bufs=1)
                oT[hi] = ps.tile([P, 512], F32, tag=f"oT{(len(oT)) % 2}",
                                 name=f"oT_{hi}", bufs=1)

                def pull():
                    try:
                        next(filler)
                    except StopIteration:
                        pass

                # iteration schedule: two-slot region one kt at a time;
                # single-slot region (kt >= Llo) in fused kt pairs
                iters = [(kt, kt) for kt in range(Llo)]
                iters += [(kt, kt + 1) for kt in range(Llo, Lhi, 2)]

                def score(it):
                    kt0, kt1 = it
                    sT2 = psS2()
                    pT = pp.tile([P, 2, 512], BF16, tag="pT", name="pT")
                    if kt0 == kt1:          # both slots at kt0
                        kt = kt0
                        cl = c0_of(lo, kt)
                        ch = c0_of(hi, kt)
                        assert ch == 0
                        nc.tensor.matmul(
                            sT2[:, 0, cl:], lhsT=kT[:, kt * P:(kt + 1) * P],
                            rhs=qT[:, q0 + cl:q0 + 512], start=True, stop=True,
                        )
                        nc.tensor.matmul(
                            sT2[:, 1, :], lhsT=kT[:, kt * P:(kt + 1) * P],
                            rhs=qT[:, q0 + 512:q0 + 1024], start=True, stop=True,
                        )
                        flat = sT2.rearrange("p a w -> p (a w)")
                        pf = pT.rearrange("p a w -> p (a w)")
                        nc.scalar.activation(pf[:, cl:], flat[:, cl:], Exp, scale=SCALE)
                        # mask windows are disjoint: lo in [0,Llo), hi in [Lhi-8,Lhi)
                        if kt >= Llo - 8:
                            # only the subtile at cl straddles the diagonal
                            nc.vector.scalar_tensor_tensor(
                                pT[:, 0, cl:cl + P], qpos_b[:, q0 + cl:q0 + cl + P],
                                kio_sb[:, kt:kt + 1], pT[:, 0, cl:cl + P],
                                op0=mybir.AluOpType.is_ge, op1=mybir.AluOpType.mult,
                            )
                        if kt == 0:
                            nc.vector.tensor_copy(dacc[:, 0:1024], pf[:])
                        else:
                            nc.vector.tensor_add(dacc[:, cl:1024], dacc[:, cl:1024],
                                                 pf[:, cl:])
                        return (pT, kt0, kt1, cl)
                    else:                   # hi only, fused kt pair
                        c0 = c0_of(hi, kt0)
                        c1 = c0_of(hi, kt1)
                        assert c0 == c1
                        for i, kt in enumerate((kt0, kt1)):
                            nc.tensor.matmul(
                                sT2[:, i, c0:], lhsT=kT[:, kt * P:(kt + 1) * P],
                                rhs=qT[:, q0 + 512 + c0:q0 + 1024],
                                start=True, stop=True,
                            )
                        nc.scalar.activation(pT[:, :, c0:], sT2[:, :, c0:], Exp,
                                             scale=SCALE)
                        if kt1 >= Lhi - 8:
                            for i, kt in enumerate((kt0, kt1)):
                                nc.vector.scalar_tensor_tensor(
                                    pT[:, i, c0:c0 + P],
                                    qpos_b[:, q0 + 512 + c0:q0 + 512 + c0 + P],
                                    kio_sb[:, kt:kt + 1], pT[:, i, c0:c0 + P],
                                    op0=mybir.AluOpType.is_ge, op1=mybir.AluOpType.mult,
                                )
                        nc.vector.tensor_add(
                            dacc[:, 512 + c0:1024], dacc[:, 512 + c0:1024],
                            pT[:, 0, c0:])
                        if kt0 == Llo:
                            nc.vector.tensor_copy(dacc[:, 1024:1536], pT[:, 1, :])
                        else:
                            nc.vector.tensor_add(
                                dacc[:, 1024 + c0:1536], dacc[:, 1024 + c0:1536],
                                pT[:, 1, c0:])
                        return (pT, kt0, kt1, None)

                def accum(st):
                    pT, kt0, kt1, cl = st
                    if kt0 == kt1:
                        kt = kt0
                        if kt < Llo:
                            nc.tensor.matmul(
                                oT[lo][:, cl:], lhsT=vN[:, kt, :], rhs=pT[:, 0, cl:],
                                start=(kt == 0), stop=(kt == Llo - 1),
                            )
                        nc.tensor.matmul(
                            oT[hi][:], lhsT=vN[:, kt, :], rhs=pT[:, 1, :],
                            start=(kt == 0), stop=(kt == Lhi - 1),
                        )
                    else:
                        c0 = c0_of(hi, kt0)
                        for i, kt in enumerate((kt0, kt1)):
                            nc.tensor.matmul(
                                oT[hi][:, c0:], lhsT=vN[:, kt, :], rhs=pT[:, i, c0:],
                                start=(kt == 0), stop=(kt == Lhi - 1),
                            )

                window = []
                for n, it in enumerate(iters):
                    window.append(score(it))
                    if n >= 4:
                        for _ in range(rate):
                            pull()
                    if len(window) > 3:
                        accum(window.pop(0))
                        if n >= 4:
                            for _ in range(rate):
                                pull()
                while window:
                    accum(window.pop(0))
                    pull()
                for _ in range(6):
                    pull()

            # ---------------- emission ----------------
            def chain(*gens):
                for g in gens:
                    yield from g

            def drain(g):
                for _ in g:
                    pass

            # pre-warm the ACT exp table during P-phase 0
            warm = epi.tile([P, 1], F32, tag="warm", name="warm")
            nc.scalar.activation(warm[:], ones_sb[:], Exp)

            # P0 in full (pass A needs vN bricks 0..7 early), but only the
            # K/Q parts of P1 gate pass A: its V rides as filler
            drain(phase_steps(0))
            drain(kq_steps(1))

            dacc_a = accp.tile([P, 1536], BF16, tag="dacc_a")
            dacc_b = accp.tile([P, 1536], BF16, tag="dacc_b")

            # P1's V plus the K/Q parts of P2/P3 ride as PE filler under
            # pass A (copies on DVE so the exp-critical ACT queue stays
            # clear).  P2/P3's V is deferred into pass B: vN bricks 16..31
            # are first consumed at kt>=16, and pass B is ACT-bound with
            # spare PE cycles.
            attention_pass(PASS_A, dacc_a,
                           chain(v_steps(1, False), kq_steps(2, False),
                                 kq_steps(3, False)), rate=3)

            # emit d/reciprocal/O-copy of slots 0,1 BEFORE pass B: pass B's
            # first accumulations WAR-wait on the oT ring slots these release,
            # so they must be ahead of pass B's DVE-queue ops (deadlock
            # avoidance), and they free the PSUM oT banks early.
            e0 = epilogue_steps(0, dacc_a, 0)
            e1 = epilogue_steps(1, dacc_a, 1, True)
            next(e0), next(e0)
            next(e1), next(e1)

            def skip(k):
                for _ in range(k):
                    yield None

            # slot 2's last accumulation pops at emission iter 26; spacers
            # keep its epilogue from being emitted before that
            attention_pass(PASS_B, dacc_b,
                           chain(v_steps(2, False), v_steps(3, False), e0, e1,
                                 skip(29), epilogue_steps(2, dacc_b, 0)))
            drain(epilogue_steps(3, dacc_b, 1, True))

    nc.compile()
    return nc


_NC_CACHE = None


def _get_nc():
    global _NC_CACHE
    if _NC_CACHE is None:
        _NC_CACHE = build_nc()
    return _NC_CACHE


def make_in_maps(inputs):
    x = np.asarray(inputs["x"], np.float32)
    Wq = np.asarray(inputs["Wq"], np.float32)
    Wk = np.asarray(inputs["Wk"], np.float32)
    Wv = np.asarray(inputs["Wv"], np.float32)
    bq = np.asarray(inputs["bq"], np.float32)

    xb = x.astype(NPBF16)

    def wshuf(W):
        # [D,H] -> [P, DCH*H]: partition p holds chunks c at rows c*128+p
        return np.ascontiguousarray(
            W.astype(NPBF16).reshape(DCH, P, H).transpose(1, 0, 2).reshape(P, DCH * H))

    common = dict(
        wq=wshuf(Wq), wk=wshuf(Wk), wv=wshuf(Wv),
        bq=bq.reshape(H, 1),
        identb=np.eye(P, dtype=NPBF16),
        onesb=np.ones((P, 1), dtype=NPBF16),
    )
    in_maps = []
    xbT = np.ascontiguousarray(xb.transpose(0, 2, 1))  # [B, D, S]
    swap = np.arange(NKT).reshape(-1, 2)[:, ::-1].reshape(-1)  # pair-swap blocks
    for c in range(8):
        b, hh = c // 2, c % 2
        m = dict(common)
        if hh == 0:
            m["xt"] = xbT[b]
            blk = np.arange(NKT)
        else:
            m["xt"] = np.ascontiguousarray(
                xbT[b].reshape(D, NKT, P)[:, swap, :].reshape(D, S))
            blk = swap
        # kio: global positions of local k-brick kt
        m["kio"] = (blk[None, :] * P + np.arange(P)[:, None] - P * hh).astype(np.int16)
        in_maps.append(m)
    return in_maps


def assemble_out(results):
    out = np.zeros((1, B, S, H), np.float32)
    for c in range(8):
        b, hh = c // 2, c % 2
        qg = qglob_for_core(hh)
        out[0, b, qg, :] = results[c]["out"].astype(np.float32)
    return out


def kernel(**inputs) -> np.ndarray:
    nc = _get_nc()
    in_maps = make_in_maps(inputs)
    res = run_bass_kernel_spmd(nc, in_maps, list(range(8)))
    out = assemble_out(res.results)
    bv = np.asarray(inputs["bv"], np.float32)
    out += (bv * SCALE)[None, None, None, :]
    return out


# revision 42
# speedup vs baseline: 1.0294x; 1.0203x over previous
"""Causal single-head attention (B=4, S=4096, D=1024, H=128) on 8 NeuronCores.

Sharding: core c = (batch b = c//2, half h = c%2). One shared SPMD program;
all per-half differences are carried in the DATA:
  - host pair-swaps adjacent 128-col blocks of x^T for h=1 cores, so a fixed
    even-local-block gather pattern selects that half's parity-interleaved
    q subtiles (global subtile g = 8r+2s+h for slot r, s in 0..3),
  - kio holds h-shifted global k positions; qpos is generated on-chip by
    iota, so is_ge(qpos, kio) is the exact causal mask.

Each core: K^T/V (full 4096 keys, replicated within the batch pair) + its own
2048 query rows.  Algebraic prunes: bk dropped (softmax shift-invariance along
k), bv folded into a host-side constant add (sum_k att = 1/sqrt(H)).

Pipeline per core (bf16 matmuls, fp32 PSUM):
  x^T host-transposed bf16, loaded as a few block DMAs with 1-2KB lines;
  weights host-preshuffled to [128, 8*128] for 2KB-line loads
  P-phases b=0..3: K/V/Q projections of stripe pair (2b,2b+1), stripe-major;
  V natural via PE transpose; Q gathered straight from resident x^T
  (strided AP); PSUM->SBUF copies ride the ACT engine
  attention in two passes, kt-outer, software-pipelined 2 deep; per kt ONE
  fused exp over both slots' adjacent PSUM banks ([P,2,512] tile);
  single-slot stretches fuse kt PAIRS into one exp.
    pass A = slots (0,1), kts 0..15, P-phases 2+3 interleaved as PE filler
    pass B = slots (2,3), kts 0..31, epilogues of slots 0,1,2 interleaved
  causal mask: one 128-col DVE is_ge STT per kt (only the subtile at the
  c0 boundary straddles the diagonal); denominator in bf16 on DVE with an
  extra accumulator per pass for the fused-pair second kt
  epilogue per slot: d^T by one-col matmuls (dacc as weights), reciprocal,
  O^T -> O by PE transpose, scale by rec*(1/sqrt(H)) via STT, bf16 DMA out
"""

import numpy as np
import ml_dtypes
from contextlib import ExitStack

import concourse.bass as bass
import concourse.tile as tile
from concourse import bacc, mybir
from concourse.bass_utils import run_bass_kernel_spmd

B, S, D, H = 4, 4096, 1024, 128
P = 128
BF16 = mybir.dt.bfloat16
F32 = mybir.dt.float32
NPBF16 = ml_dtypes.bfloat16

QLOC = 2048          # query rows per core
NKT = S // P         # 32 k tiles
DCH = D // P         # 8 contraction chunks
SCALE = 1.0 / float(np.sqrt(H))     # pre-exp scale
LIMITS = [8, 16, 24, 32]            # k-tile limit per slot (by slot id r)

# qT local column layout: natural [slot0 | slot1 | slot2 | slot3]
QOFF = {0: 0, 1: 512, 2: 1024, 3: 1536}
PASS_A = (0, 1)
PASS_B = (2, 3)


def qglob_for_core(h):
    """Global query row indices (length QLOC) in local qT order."""
    idx = []
    for r in (0, 1, 2, 3):
        for s in range(4):
            g = 8 * r + 2 * s + h
            idx.append(np.arange(g * P, (g + 1) * P))
    return np.concatenate(idx)


def c0_of(r, kt):
    """First needed column of slot r's 512-block at k-brick kt (pair-granular,
    identical for both halves)."""
    return P * max(0, (kt - 8 * r) // 2)


def build_nc():
    nc = bacc.Bacc(None, target_bir_lowering=False, debug=False, num_devices=8)

    xt = nc.dram_tensor("xt", [D, S], BF16, kind="ExternalInput").ap()
    w_ap = {}
    for nm in ("wq", "wk", "wv"):
        # host pre-shuffles to [P, DCH*H] so the load is 2KB contiguous lines
        w_ap[nm] = nc.dram_tensor(nm, [P, DCH * H], BF16, kind="ExternalInput").ap()
    bq = nc.dram_tensor("bq", [H, 1], F32, kind="ExternalInput").ap()
    kio = nc.dram_tensor("kio", [P, NKT], mybir.dt.int16, kind="ExternalInput").ap()
    identb = nc.dram_tensor("identb", [P, P], BF16, kind="ExternalInput").ap()
    onesb = nc.dram_tensor("onesb", [P, 1], BF16, kind="ExternalInput").ap()
    out = nc.dram_tensor("out", [QLOC, H], BF16, kind="ExternalOutput").ap()

    Ident = mybir.ActivationFunctionType.Identity
    Copy = mybir.ActivationFunctionType.Copy
    Exp = mybir.ActivationFunctionType.Exp

    with tile.TileContext(nc) as tc, ExitStack() as ctx:
        consts = ctx.enter_context(tc.tile_pool(name="consts", bufs=1))
        persist = ctx.enter_context(tc.tile_pool(name="persist", bufs=1))

        # ---- persistent activations
        kT = persist.tile([P, S], BF16, tag="kT")          # K^T [h, s_local]
        vN = persist.tile([P, NKT, H], BF16, tag="vN")     # V natural [k_l, kt, h]
        qT = persist.tile([P, QLOC], BF16, tag="qT")       # Q^T [h, q_local]
        xt_sb = persist.tile([P, DCH, S], BF16, tag="xt_sb")    # x^T resident
        # gather view: S = (pair b:4)(s:4)(parity:2)(w:128)
        xt_g = xt_sb.rearrange("p c (b s two w) -> p c b s two w", b=4, s=4, two=2)

        # ---- loads: wk + stripe0/1 first (gate the first projections), the
        # rest of x^T as 2MB blocks (2KB lines, one descriptor set each)
        xt_r = xt.rearrange("(c p) s -> p c s", p=P)

        def load_w(nm, eng):
            t = consts.tile([P, DCH, H], BF16, tag=f"w_{nm}", name=f"w_{nm}")
            eng.dma_start(out=t[:], in_=w_ap[nm].rearrange("p (c h) -> p c h", c=DCH))
            return t

        def load_x_cols(c0, c1):
            nc.sync.dma_start(out=xt_sb[:, :, c0:c1], in_=xt_r[:, :, c0:c1])

        # parallel descriptor generation across the three DMA-capable queues:
        # sync carries wk + x stripes, scalar carries wv/wq + stripe 1,
        # gpsimd carries the small consts
        w_sb = {}
        w_sb["wk"] = load_w("wk", nc.sync)
        for c4 in range(0, 8, 2):
            nc.sync.dma_start(out=xt_sb[:, c4:c4 + 2, 0:512],
                              in_=xt_r[:, c4:c4 + 2, 0:512])
        w_sb["wv"] = load_w("wv", nc.scalar)
        load_x_cols(512, 1024)
        w_sb["wq"] = load_w("wq", nc.scalar)
        load_x_cols(1024, 2048)
        load_x_cols(2048, 3072)
        load_x_cols(3072, 4096)

        bq_sb = consts.tile([P, 1], F32, tag="bq")
        nc.gpsimd.dma_start(out=bq_sb[:], in_=bq)
        identb_sb = consts.tile([P, P], BF16, tag="identb")
        nc.gpsimd.dma_start(out=identb_sb[:], in_=identb)
        ones_sb = consts.tile([P, 1], BF16, tag="ones")
        nc.gpsimd.dma_start(out=ones_sb[:], in_=onesb)
        kio_sb = consts.tile([P, NKT], mybir.dt.int16, tag="kio")
        nc.gpsimd.dma_start(out=kio_sb[:], in_=kio)

        # qpos generated on-chip in h-shifted coordinates (global - 128h):
        # affine per 512-block (base 1024*slot + 256*s + w); the host shifts
        # kio by -128h to match, so is_ge(qpos, kio) is exact causal.
        qpos_b = consts.tile([P, QLOC], mybir.dt.int16, tag="qpos_b")
        for blk, r in enumerate((0, 1, 2, 3)):
            nc.gpsimd.iota(qpos_b[:, blk * 512:(blk + 1) * 512],
                           pattern=[[256, 4], [1, P]], base=1024 * r,
                           channel_multiplier=0)

        with tc.tile_pool(name="ps", bufs=1, space="PSUM") as ps, \
             tc.tile_pool(name="stg", bufs=3) as stg, \
             tc.tile_pool(name="pp", bufs=8) as pp, \
             tc.tile_pool(name="acc", bufs=1) as accp, \
             tc.tile_pool(name="epi", bufs=3) as epi:

            # PSUM budget (8 banks): mm512 x2 + sT2 (2 banks) x2 + oT a/b = 8
            def psA():
                return ps.tile([P, 512], F32, tag="mm512", name="mm512", bufs=2)

            def psA_b16():
                return ps.tile([P, 512], BF16, tag="mm512", name="mm512b", bufs=2)

            def psS2():
                return ps.tile([P, 2, 512], F32, tag="sT2", name="sT2", bufs=2)

            oT = {}   # slot -> PSUM tile [P, 512] f32, 2 live at a time

            # ---------------- projections ----------------
            def proj_stripe_mm(wname, sr, ps_t):
                for j in range(DCH):
                    nc.tensor.matmul(
                        ps_t[:], lhsT=w_sb[wname][:, j, :],
                        rhs=xt_sb[:, j, sr * 512:(sr + 1) * 512],
                        start=(j == 0), stop=(j == DCH - 1),
                    )
                    if j == 3:
                        yield None
                yield None

            def copy_fn(on_act):
                def copy(dst, src, bias=None):
                    if on_act:
                        nc.scalar.activation(dst, src, Ident if bias is not None
                                             else Copy,
                                             **({"bias": bias, "scale": 1.0}
                                                if bias is not None else {}))
                    elif bias is not None:
                        nc.vector.tensor_scalar_add(dst, src, bias)
                    else:
                        nc.vector.tensor_copy(dst, src)
                return copy

            def kq_steps(b, on_act=True):
                """K projections of stripes 2b,2b+1 + Q of slot b (the parts
                a pass is gated on).  on_act: copies on ACT (idle in P-phases);
                else DVE (for filler inside a pass where ACT runs exps)."""
                copy = copy_fn(on_act)
                for sr in (2 * b, 2 * b + 1):
                    pk = psA()
                    yield from proj_stripe_mm("wk", sr, pk)
                    copy(kT[:, sr * 512:(sr + 1) * 512], pk[:])
                # Q projection for slot r=b (gather even local blocks of the pair)
                pq = psA()
                for j in range(DCH):
                    nc.tensor.matmul(
                        pq[:], lhsT=w_sb["wq"][:, j, :],
                        rhs=xt_g[:, j, b, :, 0, :],
                        start=(j == 0), stop=(j == DCH - 1),
                    )
                    if j == 3:
                        yield None
                q0 = QOFF[b]
                copy(qT[:, q0:q0 + 512], pq[:], bias=bq_sb[:])
                yield None

            def v_steps(b, on_act=True):
                """V projections of stripes 2b,2b+1 + PE transposes to vN."""
                copy = copy_fn(on_act)
                vTs = stg.tile([P, 1024], BF16, tag="vTs", name="vTs")
                for i, sr in enumerate((2 * b, 2 * b + 1)):
                    pv = psA()
                    yield from proj_stripe_mm("wv", sr, pv)
                    copy(vTs[:, i * 512:(i + 1) * 512], pv[:])
                for half in range(2):
                    pst = psA_b16()
                    for t_ in range(4):
                        nc.tensor.matmul(
                            pst[:, t_ * P:(t_ + 1) * P],
                            lhsT=vTs[:, half * 512 + t_ * P: half * 512 + (t_ + 1) * P],
                            rhs=identb_sb[:], is_transpose=True, skip_group_check=True,
                        )
                    nc.vector.tensor_copy(
                        vN[:, (2 * b + half) * 4:(2 * b + half + 1) * 4, :], pst[:]
                    )
                    yield None

            def phase_steps(b, on_act=True):
                """Full phase, stripe-major interleaved (K s, V s alternate) so
                a pending second-stripe DMA hides under first-stripe V work."""
                copy = copy_fn(on_act)
                vTs = stg.tile([P, 1024], BF16, tag="vTs", name="vTs")
                for i, sr in enumerate((2 * b, 2 * b + 1)):
                    pk = psA()
                    yield from proj_stripe_mm("wk", sr, pk)
                    copy(kT[:, sr * 512:(sr + 1) * 512], pk[:])
                    pv = psA()
                    yield from proj_stripe_mm("wv", sr, pv)
                    copy(vTs[:, i * 512:(i + 1) * 512], pv[:])
                pq = psA()
                for j in range(DCH):
                    nc.tensor.matmul(
                        pq[:], lhsT=w_sb["wq"][:, j, :],
                        rhs=xt_g[:, j, b, :, 0, :],
                        start=(j == 0), stop=(j == DCH - 1),
                    )
                    if j == 3:
                        yield None
                q0 = QOFF[b]
                copy(qT[:, q0:q0 + 512], pq[:], bias=bq_sb[:])
                yield None
                for half in range(2):
                    pst = psA_b16()
                    for t_ in range(4):
                        nc.tensor.matmul(
                            pst[:, t_ * P:(t_ + 1) * P],
                            lhsT=vTs[:, half * 512 + t_ * P: half * 512 + (t_ + 1) * P],
                            rhs=identb_sb[:], is_transpose=True, skip_group_check=True,
                        )
                    nc.vector.tensor_copy(
                        vN[:, (2 * b + half) * 4:(2 * b + half + 1) * 4, :], pst[:]
                    )
                    yield None

            # ---------------- epilogue ----------------
            def epilogue_steps(r, dacc, di, dacc2=False):
                """Generator: output slot r.  dacc [P,1024] bf16, di = 0/1 half."""
                oTs = epi.tile([P, 512], BF16, tag="oTs", name="oTs")
                nc.vector.tensor_copy(oTs[:], oT[r][:])
                dT = psA()
                for s_ in range(4):
                    srcs = [dacc[:, di * 512 + s_ * P: di * 512 + (s_ + 1) * P]]
                    if dacc2:
                        srcs.append(dacc[:, 1024 + s_ * P: 1024 + (s_ + 1) * P])
                    for k_, sc in enumerate(srcs):
                        nc.tensor.matmul(
                            dT[:, s_:s_ + 1], lhsT=sc, rhs=ones_sb[:],
                            start=(k_ == 0), stop=(k_ == len(srcs) - 1),
                            skip_group_check=True,
                        )
                yield None
                rec = epi.tile([P, 4], F32, tag="rec", name="rec")
                nc.vector.reciprocal(rec[:], dT[:, 0:4])
                yield None
                obr = psA_b16()
                for s_ in range(4):
                    nc.tensor.matmul(
                        obr[:, s_ * P:(s_ + 1) * P], lhsT=oTs[:, s_ * P:(s_ + 1) * P],
                        rhs=identb_sb[:], is_transpose=True, skip_group_check=True,
                    )
                yield None
                ofin = epi.tile([P, 512], BF16, tag="ofin", name="ofin")
                rec_b = bass.AP(tensor=rec.tensor, offset=rec.offset,
                                ap=[rec.ap[0], [1, 4], [0, P]])
                nc.vector.scalar_tensor_tensor(
                    ofin[:].rearrange("p (s w) -> p s w", s=4),
                    obr[:].rearrange("p (s w) -> p s w", s=4),
                    SCALE, rec_b,
                    op0=mybir.AluOpType.mult, op1=mybir.AluOpType.mult)
                yield None
                q0 = QOFF[r]
                nc.sync.dma_start(
                    out=out[q0:q0 + 512, :].rearrange("(s p) h -> p s h", p=P),
                    in_=ofin[:].rearrange("p (s h) -> p s h", s=4),
                )
                yield None

            # ---------------- attention pass ----------------
            def attention_pass(slots, dacc, filler, rate=1):
                """kt-outer pass over slot pair (lo, hi); lo has smaller limit.
                qT cols: lo at QOFF[lo], hi at QOFF[hi] (adjacent, lo first).
                filler: iterator of generators for PE filler work."""
                lo, hi = slots
                Llo, Lhi = LIMITS[lo], LIMITS[hi]
                q0 = QOFF[lo]
                assert QOFF[hi] == q0 + 512
                oT[lo] = ps.tile([P, 512], F32, tag=f"oT{len(oT) % 2}",
                                 name=f"oT_{lo}", bufs=1)
                oT[hi] = ps.tile([P, 512], F32, tag=f"oT{(len(oT)) % 2}",
                                 name=f"oT_{hi}", bufs=1)

                def pull():
                    try:
                        next(filler)
                    except StopIteration:
                        pass

                # iteration schedule: two-slot region one kt at a time;
                # single-slot region (kt >= Llo) in fused kt pairs
                iters = [(kt, kt) for kt in range(Llo)]
                iters += [(kt, kt + 1) for kt in range(Llo, Lhi, 2)]

                def score(it):
                    kt0, kt1 = it
                    sT2 = psS2()
                    pT = pp.tile([P, 2, 512], BF16, tag="pT", name="pT")
                    if kt0 == kt1:          # both slots at kt0
                        kt = kt0
                        cl = c0_of(lo, kt)
                        ch = c0_of(hi, kt)
                        assert ch == 0
                        nc.tensor.matmul(
                            sT2[:, 0, cl:], lhsT=kT[:, kt * P:(kt + 1) * P],
                            rhs=qT[:, q0 + cl:q0 + 512], start=True, stop=True,
                        )
                        nc.tensor.matmul(
                            sT2[:, 1, :], lhsT=kT[:, kt * P:(kt + 1) * P],
                            rhs=qT[:, q0 + 512:q0 + 1024], start=True, stop=True,
                        )
                        flat = sT2.rearrange("p a w -> p (a w)")
                        pf = pT.rearrange("p a w -> p (a w)")
                        nc.scalar.activation(pf[:, cl:], flat[:, cl:], Exp, scale=SCALE)
                        # mask windows are disjoint: lo in [0,Llo), hi in [Lhi-8,Lhi)
                        if kt >= Llo - 8:
                            # only the subtile at cl straddles the diagonal
                            nc.vector.scalar_tensor_tensor(
                                pT[:, 0, cl:cl + P], qpos_b[:, q0 + cl:q0 + cl + P],
                                kio_sb[:, kt:kt + 1], pT[:, 0, cl:cl + P],
                                op0=mybir.AluOpType.is_ge, op1=mybir.AluOpType.mult,
                            )
                        if kt == 0:
                            nc.vector.tensor_copy(dacc[:, 0:1024], pf[:])
                        else:
                            nc.vector.tensor_add(dacc[:, cl:1024], dacc[:, cl:1024],
                                                 pf[:, cl:])
                        return (pT, kt0, kt1, cl)
                    else:                   # hi only, fused kt pair
                        c0 = c0_of(hi, kt0)
                        c1 = c0_of(hi, kt1)
                        assert c0 == c1
                        for i, kt in enumerate((kt0, kt1)):
                            nc.tensor.matmul(
                                sT2[:, i, c0:], lhsT=kT[:, kt * P:(kt + 1) * P],
                                rhs=qT[:, q0 + 512 + c0:q0 + 1024],
                                start=True, stop=True,
                            )
                        nc.scalar.activation(pT[:, :, c0:], sT2[:, :, c0:], Exp,
                                             scale=SCALE)
                        if kt1 >= Lhi - 8:
                            for i, kt in enumerate((kt0, kt1)):
                                nc.vector.scalar_tensor_tensor(
                                    pT[:, i, c0:c0 + P],
                                    qpos_b[:, q0 + 512 + c0:q0 + 512 + c0 + P],
                                    kio_sb[:, kt:kt + 1], pT[:, i, c0:c0 + P],
                                    op0=mybir.AluOpType.is_ge, op1=mybir.AluOpType.mult,
                                )
                        nc.vector.tensor_add(
                            dacc[:, 512 + c0:1024], dacc[:, 512 + c0:1024],
                            pT[:, 0, c0:])
                        if kt0 == Llo:
                            nc.vector.tensor_copy(dacc[:, 1024:1536], pT[:, 1, :])
                        else:
                            nc.vector.tensor_add(
                                dacc[:, 1024 + c0:1536], dacc[:, 1024 + c0:1536],
                                pT[:, 1, c0:])
                        return (pT, kt0, kt1, None)

                def accum(st):
                    pT, kt0, kt1, cl = st
                    if kt0 == kt1:
                        kt = kt0
                        if kt < Llo:
                            nc.tensor.matmul(
                                oT[lo][:, cl:], lhsT=vN[:, kt, :], rhs=pT[:, 0, cl:],
                                start=(kt == 0), stop=(kt == Llo - 1),
                            )
                        nc.tensor.matmul(
                            oT[hi][:], lhsT=vN[:, kt, :], rhs=pT[:, 1, :],
                            start=(kt == 0), stop=(kt == Lhi - 1),
                        )
                    else:
                        c0 = c0_of(hi, kt0)
                        for i, kt in enumerate((kt0, kt1)):
                            nc.tensor.matmul(
                                oT[hi][:, c0:], lhsT=vN[:, kt, :], rhs=pT[:, i, c0:],
                                start=(kt == 0), stop=(kt == Lhi - 1),
                            )

                window = []
                for n, it in enumerate(iters):
                    window.append(score(it))
                    if n >= 4:
                        for _ in range(rate):
                            pull()
                    if len(window) > 3:
                        accum(window.pop(0))
                        if n >= 4:
                            for _ in range(rate):
                                pull()
                while window:
                    accum(window.pop(0))
                    pull()
                for _ in range(6):
                    pull()

            # ---------------- emission ----------------
            def chain(*gens):
                for g in gens:
                    yield from g

            def drain(g):
                for _ in g:
                    pass

            # pre-warm the ACT exp table during P-phase 0
            warm = epi.tile([P, 1], F32, tag="warm", name="warm")
            nc.scalar.activation(warm[:], ones_sb[:], Exp)

            # P0 in full (pass A needs vN bricks 0..7 early), but only the
            # K/Q parts of P1 gate pass A: its V rides as filler
            drain(phase_steps(0))
            drain(kq_steps(1))

            dacc_a = accp.tile([P, 1536], BF16, tag="dacc_a")
            dacc_b = accp.tile([P, 1536], BF16, tag="dacc_b")

            # P1's V plus the K/Q parts of P2/P3 ride as PE filler under
            # pass A (copies on DVE so the exp-critical ACT queue stays
            # clear).  P2/P3's V is deferred into pass B: vN bricks 16..31
            # are first consumed at kt>=16, and pass B is ACT-bound with
            # spare PE cycles.
            attention_pass(PASS_A, dacc_a,
                           chain(v_steps(1, False), kq_steps(2, False),
                                 kq_steps(3, False)), rate=3)

            # emit d/reciprocal/O-copy of slots 0,1 BEFORE pass B: pass B's
            # first accumulations WAR-wait on the oT ring slots these release,
            # so they must be ahead of pass B's DVE-queue ops (deadlock
            # avoidance), and they free the PSUM oT banks early.
            e0 = epilogue_steps(0, dacc_a, 0)
            e1 = epilogue_steps(1, dacc_a, 1, True)
            next(e0), next(e0)
            next(e1), next(e1)

            def skip(k):
                for _ in range(k):
                    yield None

            # slot 2's last accumulation pops at emission iter 26; spacers
            # keep its epilogue from being emitted before that
            attention_pass(PASS_B, dacc_b,
                           chain(v_steps(2, False), v_steps(3, False), e0, e1,
                                 skip(29), epilogue_steps(2, dacc_b, 0)))
            drain(epilogue_steps(3, dacc_b, 1, True))

    nc.compile()
    return nc


_NC_CACHE = None


def _get_nc():
    global _NC_CACHE
    if _NC_CACHE is None:
        _NC_CACHE = build_nc()
    return _NC_CACHE


def make_in_maps(inputs):
    x = np.asarray(inputs["x"], np.float32)
    Wq = np.asarray(inputs["Wq"], np.float32)
    Wk = np.asarray(inputs["Wk"], np.float32)
    Wv = np.asarray(inputs["Wv"], np.float32)
    bq = np.asarray(inputs["bq"], np.float32)

    xb = x.astype(NPBF16)

    def wshuf(W):
        # [D,H] -> [P, DCH*H]: partition p holds chunks c at rows c*128+p
        return np.ascontiguousarray(
            W.astype(NPBF16).reshape(DCH, P, H).transpose(1, 0, 2).reshape(P, DCH * H))

    common = dict(
        wq=wshuf(Wq), wk=wshuf(Wk), wv=wshuf(Wv),
        bq=bq.reshape(H, 1),
        identb=np.eye(P, dtype=NPBF16),
        onesb=np.ones((P, 1), dtype=NPBF16),
    )
    in_maps = []
    xbT = np.ascontiguousarray(xb.transpose(0, 2, 1))  # [B, D, S]
    swap = np.arange(NKT).reshape(-1, 2)[:, ::-1].reshape(-1)  # pair-swap blocks
    for c in range(8):
        b, hh = c // 2, c % 2
        m = dict(common)
        if hh == 0:
            m["xt"] = xbT[b]
            blk = np.arange(NKT)
        else:
            m["xt"] = np.ascontiguousarray(
                xbT[b].reshape(D, NKT, P)[:, swap, :].reshape(D, S))
            blk = swap
        # kio: global positions of local k-brick kt
        m["kio"] = (blk[None, :] * P + np.arange(P)[:, None] - P * hh).astype(np.int16)
        in_maps.append(m)
    return in_maps


def assemble_out(results):
    out = np.zeros((1, B, S, H), np.float32)
    for c in range(8):
        b, hh = c // 2, c % 2
        qg = qglob_for_core(hh)
        out[0, b, qg, :] = results[c]["out"].astype(np.float32)
    return out


def kernel(**inputs) -> np.ndarray:
    nc = _get_nc()
    in_maps = make_in_maps(inputs)
    res = run_bass_kernel_spmd(nc, in_maps, list(range(8)))
    out = assemble_out(res.results)
    bv = np.asarray(inputs["bv"], np.float32)
    out += (bv * SCALE)[None, None, None, :]
    return out


# revision 43
# speedup vs baseline: 1.0332x; 1.0038x over previous
"""Causal single-head attention (B=4, S=4096, D=1024, H=128) on 8 NeuronCores.

Sharding: core c = (batch b = c//2, half h = c%2). One shared SPMD program;
all per-half differences are carried in the DATA:
  - host pair-swaps adjacent 128-col blocks of x^T for h=1 cores, so a fixed
    even-local-block gather pattern selects that half's parity-interleaved
    q subtiles (global subtile g = 8r+2s+h for slot r, s in 0..3),
  - kio holds h-shifted global k positions; qpos is generated on-chip by
    iota, so is_ge(qpos, kio) is the exact causal mask.

Each core: K^T/V (full 4096 keys, replicated within the batch pair) + its own
2048 query rows.  Algebraic prunes: bk dropped (softmax shift-invariance along
k), bv folded into a host-side constant add (sum_k att = 1/sqrt(H)).

Pipeline per core (bf16 matmuls, fp32 PSUM):
  x^T host-transposed bf16, loaded as a few block DMAs with 1-2KB lines;
  weights host-preshuffled to [128, 8*128] for 2KB-line loads
  P-phases b=0..3: K/V/Q projections of stripe pair (2b,2b+1), stripe-major;
  V natural via PE transpose; Q gathered straight from resident x^T
  (strided AP); PSUM->SBUF copies ride the ACT engine
  attention in two passes, kt-outer, software-pipelined 2 deep; per kt ONE
  fused exp over both slots' adjacent PSUM banks ([P,2,512] tile);
  single-slot stretches fuse kt PAIRS into one exp.
    pass A = slots (0,1), kts 0..15, P-phases 2+3 interleaved as PE filler
    pass B = slots (2,3), kts 0..31, epilogues of slots 0,1,2 interleaved
  causal mask: one 128-col DVE is_ge STT per kt (only the subtile at the
  c0 boundary straddles the diagonal); denominator in bf16 on DVE with an
  extra accumulator per pass for the fused-pair second kt
  epilogue per slot: d^T by one-col matmuls (dacc as weights), reciprocal,
  O^T -> O by PE transpose, scale by rec*(1/sqrt(H)) via STT, bf16 DMA out
"""

import numpy as np
import ml_dtypes
from contextlib import ExitStack

import concourse.bass as bass
import concourse.tile as tile
from concourse import bacc, mybir
from concourse.bass_utils import run_bass_kernel_spmd

B, S, D, H = 4, 4096, 1024, 128
P = 128
BF16 = mybir.dt.bfloat16
F32 = mybir.dt.float32
NPBF16 = ml_dtypes.bfloat16

QLOC = 2048          # query rows per core
NKT = S // P         # 32 k tiles
DCH = D // P         # 8 contraction chunks
SCALE = 1.0 / float(np.sqrt(H))     # pre-exp scale
LIMITS = [8, 16, 24, 32]            # k-tile limit per slot (by slot id r)

# qT local column layout: natural [slot0 | slot1 | slot2 | slot3]
QOFF = {0: 0, 1: 512, 2: 1024, 3: 1536}
PASS_A = (0, 1)
PASS_B = (2, 3)


def qglob_for_core(h):
    """Global query row indices (length QLOC) in local qT order."""
    idx = []
    for r in (0, 1, 2, 3):
        for s in range(4):
            g = 8 * r + 2 * s + h
            idx.append(np.arange(g * P, (g + 1) * P))
    return np.concatenate(idx)


def c0_of(r, kt):
    """First needed column of slot r's 512-block at k-brick kt (pair-granular,
    identical for both halves)."""
    return P * max(0, (kt - 8 * r) // 2)


def build_nc():
    nc = bacc.Bacc(None, target_bir_lowering=False, debug=False, num_devices=8)

    xt = nc.dram_tensor("xt", [D, S], BF16, kind="ExternalInput").ap()
    w_ap = {}
    for nm in ("wq", "wk", "wv"):
        # host pre-shuffles to [P, DCH*H] so the load is 2KB contiguous lines
        w_ap[nm] = nc.dram_tensor(nm, [P, DCH * H], BF16, kind="ExternalInput").ap()
    bq = nc.dram_tensor("bq", [H, 1], F32, kind="ExternalInput").ap()
    kio = nc.dram_tensor("kio", [P, NKT], mybir.dt.int16, kind="ExternalInput").ap()
    identb = nc.dram_tensor("identb", [P, P], BF16, kind="ExternalInput").ap()
    onesb = nc.dram_tensor("onesb", [P, 1], BF16, kind="ExternalInput").ap()
    out = nc.dram_tensor("out", [QLOC, H], BF16, kind="ExternalOutput").ap()

    Ident = mybir.ActivationFunctionType.Identity
    Copy = mybir.ActivationFunctionType.Copy
    Exp = mybir.ActivationFunctionType.Exp

    with tile.TileContext(nc) as tc, ExitStack() as ctx:
        consts = ctx.enter_context(tc.tile_pool(name="consts", bufs=1))
        persist = ctx.enter_context(tc.tile_pool(name="persist", bufs=1))

        # ---- persistent activations
        kT = persist.tile([P, S], BF16, tag="kT")          # K^T [h, s_local]
        vN = persist.tile([P, NKT, H], BF16, tag="vN")     # V natural [k_l, kt, h]
        qT = persist.tile([P, QLOC], BF16, tag="qT")       # Q^T [h, q_local]
        xt_sb = persist.tile([P, DCH, S], BF16, tag="xt_sb")    # x^T resident
        # gather view: S = (pair b:4)(s:4)(parity:2)(w:128)
        xt_g = xt_sb.rearrange("p c (b s two w) -> p c b s two w", b=4, s=4, two=2)

        # ---- loads: wk + stripe0/1 first (gate the first projections), the
        # rest of x^T as 2MB blocks (2KB lines, one descriptor set each)
        xt_r = xt.rearrange("(c p) s -> p c s", p=P)

        def load_w(nm, eng):
            t = consts.tile([P, DCH, H], BF16, tag=f"w_{nm}", name=f"w_{nm}")
            eng.dma_start(out=t[:], in_=w_ap[nm].rearrange("p (c h) -> p c h", c=DCH))
            return t

        def load_x_cols(c0, c1):
            nc.sync.dma_start(out=xt_sb[:, :, c0:c1], in_=xt_r[:, :, c0:c1])

        # parallel descriptor generation across the three DMA-capable queues:
        # sync carries wk + x stripes, scalar carries wv/wq + stripe 1,
        # gpsimd carries the small consts
        w_sb = {}
        w_sb["wk"] = load_w("wk", nc.sync)
        for c4 in range(0, 8, 2):
            nc.sync.dma_start(out=xt_sb[:, c4:c4 + 2, 0:512],
                              in_=xt_r[:, c4:c4 + 2, 0:512])
        w_sb["wv"] = load_w("wv", nc.scalar)
        load_x_cols(512, 1024)
        w_sb["wq"] = load_w("wq", nc.scalar)
        load_x_cols(1024, 2048)
        load_x_cols(2048, 3072)
        load_x_cols(3072, 4096)

        bq_sb = consts.tile([P, 1], F32, tag="bq")
        nc.gpsimd.dma_start(out=bq_sb[:], in_=bq)
        identb_sb = consts.tile([P, P], BF16, tag="identb")
        nc.gpsimd.dma_start(out=identb_sb[:], in_=identb)
        ones_sb = consts.tile([P, 1], BF16, tag="ones")
        nc.gpsimd.dma_start(out=ones_sb[:], in_=onesb)
        kio_sb = consts.tile([P, NKT], mybir.dt.int16, tag="kio")
        nc.gpsimd.dma_start(out=kio_sb[:], in_=kio)

        # qpos generated on-chip in h-shifted coordinates (global - 128h):
        # affine per 512-block (base 1024*slot + 256*s + w); the host shifts
        # kio by -128h to match, so is_ge(qpos, kio) is exact causal.
        qpos_b = consts.tile([P, QLOC], mybir.dt.int16, tag="qpos_b")
        for blk, r in enumerate((0, 1, 2, 3)):
            nc.gpsimd.iota(qpos_b[:, blk * 512:(blk + 1) * 512],
                           pattern=[[256, 4], [1, P]], base=1024 * r,
                           channel_multiplier=0)

        with tc.tile_pool(name="ps", bufs=1, space="PSUM") as ps, \
             tc.tile_pool(name="stg", bufs=3) as stg, \
             tc.tile_pool(name="pp", bufs=8) as pp, \
             tc.tile_pool(name="acc", bufs=1) as accp, \
             tc.tile_pool(name="epi", bufs=3) as epi:

            # PSUM budget (8 banks): mm512 x2 + sT2 (2 banks) x2 + oT a/b = 8
            def psA():
                return ps.tile([P, 512], F32, tag="mm512", name="mm512", bufs=2)

            def psA_b16():
                return ps.tile([P, 512], BF16, tag="mm512", name="mm512b", bufs=2)

            def psS2():
                return ps.tile([P, 2, 512], F32, tag="sT2", name="sT2", bufs=2)

            oT = {}   # slot -> PSUM tile [P, 512] f32, 2 live at a time

            # ---------------- projections ----------------
            def proj_stripe_mm(wname, sr, ps_t):
                for j in range(DCH):
                    nc.tensor.matmul(
                        ps_t[:], lhsT=w_sb[wname][:, j, :],
                        rhs=xt_sb[:, j, sr * 512:(sr + 1) * 512],
                        start=(j == 0), stop=(j == DCH - 1),
                    )
                    if j == 3:
                        yield None
                yield None

            def copy_fn(on_act):
                def copy(dst, src, bias=None):
                    if on_act:
                        nc.scalar.activation(dst, src, Ident if bias is not None
                                             else Copy,
                                             **({"bias": bias, "scale": 1.0}
                                                if bias is not None else {}))
                    elif bias is not None:
                        nc.vector.tensor_scalar_add(dst, src, bias)
                    else:
                        nc.vector.tensor_copy(dst, src)
                return copy

            def kq_steps(b, on_act=True):
                """K projections of stripes 2b,2b+1 + Q of slot b (the parts
                a pass is gated on).  on_act: copies on ACT (idle in P-phases);
                else DVE (for filler inside a pass where ACT runs exps)."""
                copy = copy_fn(on_act)
                for sr in (2 * b, 2 * b + 1):
                    pk = psA()
                    yield from proj_stripe_mm("wk", sr, pk)
                    copy(kT[:, sr * 512:(sr + 1) * 512], pk[:])
                # Q projection for slot r=b (gather even local blocks of the pair)
                pq = psA()
                for j in range(DCH):
                    nc.tensor.matmul(
                        pq[:], lhsT=w_sb["wq"][:, j, :],
                        rhs=xt_g[:, j, b, :, 0, :],
                        start=(j == 0), stop=(j == DCH - 1),
                    )
                    if j == 3:
                        yield None
                q0 = QOFF[b]
                copy(qT[:, q0:q0 + 512], pq[:], bias=bq_sb[:])
                yield None

            def v_steps(b, on_act=True):
                """V projections of stripes 2b,2b+1 + PE transposes to vN."""
                copy = copy_fn(on_act)
                vTs = stg.tile([P, 1024], BF16, tag="vTs", name="vTs")
                for i, sr in enumerate((2 * b, 2 * b + 1)):
                    pv = psA()
                    yield from proj_stripe_mm("wv", sr, pv)
                    copy(vTs[:, i * 512:(i + 1) * 512], pv[:])
                for half in range(2):
                    pst = psA_b16()
                    for t_ in range(4):
                        nc.tensor.matmul(
                            pst[:, t_ * P:(t_ + 1) * P],
                            lhsT=vTs[:, half * 512 + t_ * P: half * 512 + (t_ + 1) * P],
                            rhs=identb_sb[:], is_transpose=True, skip_group_check=True,
                        )
                    nc.vector.tensor_copy(
                        vN[:, (2 * b + half) * 4:(2 * b + half + 1) * 4, :], pst[:]
                    )
                    yield None

            def phase_steps(b, on_act=True):
                """Full phase, stripe-major interleaved (K s, V s alternate) so
                a pending second-stripe DMA hides under first-stripe V work."""
                copy = copy_fn(on_act)
                vTs = stg.tile([P, 1024], BF16, tag="vTs", name="vTs")
                for i, sr in enumerate((2 * b, 2 * b + 1)):
                    pk = psA()
                    yield from proj_stripe_mm("wk", sr, pk)
                    copy(kT[:, sr * 512:(sr + 1) * 512], pk[:])
                    pv = psA()
                    yield from proj_stripe_mm("wv", sr, pv)
                    copy(vTs[:, i * 512:(i + 1) * 512], pv[:])
                pq = psA()
                for j in range(DCH):
                    nc.tensor.matmul(
                        pq[:], lhsT=w_sb["wq"][:, j, :],
                        rhs=xt_g[:, j, b, :, 0, :],
                        start=(j == 0), stop=(j == DCH - 1),
                    )
                    if j == 3:
                        yield None
                q0 = QOFF[b]
                copy(qT[:, q0:q0 + 512], pq[:], bias=bq_sb[:])
                yield None
                for half in range(2):
                    pst = psA_b16()
                    for t_ in range(4):
                        nc.tensor.matmul(
                            pst[:, t_ * P:(t_ + 1) * P],
                            lhsT=vTs[:, half * 512 + t_ * P: half * 512 + (t_ + 1) * P],
                            rhs=identb_sb[:], is_transpose=True, skip_group_check=True,
                        )
                    nc.vector.tensor_copy(
                        vN[:, (2 * b + half) * 4:(2 * b + half + 1) * 4, :], pst[:]
                    )
                    yield None

            # ---------------- epilogue ----------------
            def epilogue_steps(r, dacc, di, dacc2=False, split=False):
                """Generator: output slot r.  dacc [P,1024] bf16, di = 0/1 half.
                split: process in two column-halves so the first half's DMA
                overlaps the second half's compute (for the tail slot)."""
                oTs = epi.tile([P, 512], BF16, tag="oTs", name="oTs")
                if split:
                    nc.vector.tensor_copy(oTs[:, 0:256], oT[r][:, 0:256])
                else:
                    nc.vector.tensor_copy(oTs[:], oT[r][:])
                dT = psA()
                for s_ in range(4):
                    srcs = [dacc[:, di * 512 + s_ * P: di * 512 + (s_ + 1) * P]]
                    if dacc2:
                        srcs.append(dacc[:, 1024 + s_ * P: 1024 + (s_ + 1) * P])
                    for k_, sc in enumerate(srcs):
                        nc.tensor.matmul(
                            dT[:, s_:s_ + 1], lhsT=sc, rhs=ones_sb[:],
                            start=(k_ == 0), stop=(k_ == len(srcs) - 1),
                            skip_group_check=True,
                        )
                yield None
                rec = epi.tile([P, 4], F32, tag="rec", name="rec")
                nc.vector.reciprocal(rec[:], dT[:, 0:4])
                yield None
                q0 = QOFF[r]
                obr = psA_b16()
                ofin = epi.tile([P, 512], BF16, tag="ofin", name="ofin")
                halves = (range(0, 2), range(2, 4)) if split else (range(0, 4),)
                for hi_, ss in enumerate(halves):
                    if split and hi_ == 1:
                        nc.vector.tensor_copy(oTs[:, 256:512], oT[r][:, 256:512])
                    for s_ in ss:
                        nc.tensor.matmul(
                            obr[:, s_ * P:(s_ + 1) * P],
                            lhsT=oTs[:, s_ * P:(s_ + 1) * P],
                            rhs=identb_sb[:], is_transpose=True,
                            skip_group_check=True,
                        )
                    a, b_ = ss[0] * P, (ss[-1] + 1) * P
                    rec_b = bass.AP(tensor=rec.tensor, offset=rec.offset + ss[0],
                                    ap=[rec.ap[0], [1, len(ss)], [0, P]])
                    nc.vector.scalar_tensor_tensor(
                        ofin[:, a:b_].rearrange("p (s w) -> p s w", s=len(ss)),
                        obr[:, a:b_].rearrange("p (s w) -> p s w", s=len(ss)),
                        SCALE, rec_b,
                        op0=mybir.AluOpType.mult, op1=mybir.AluOpType.mult)
                    nc.sync.dma_start(
                        out=out[q0 + ss[0] * P:q0 + (ss[-1] + 1) * P, :]
                        .rearrange("(s p) h -> p s h", p=P),
                        in_=ofin[:, a:b_].rearrange("p (s h) -> p s h", s=len(ss)),
                    )
                    yield None
                yield None

            # ---------------- attention pass ----------------
            def attention_pass(slots, dacc, filler, rate=1):
                """kt-outer pass over slot pair (lo, hi); lo has smaller limit.
                qT cols: lo at QOFF[lo], hi at QOFF[hi] (adjacent, lo first).
                filler: iterator of generators for PE filler work."""
                lo, hi = slots
                Llo, Lhi = LIMITS[lo], LIMITS[hi]
                q0 = QOFF[lo]
                assert QOFF[hi] == q0 + 512
                oT[lo] = ps.tile([P, 512], F32, tag=f"oT{len(oT) % 2}",
                                 name=f"oT_{lo}", bufs=1)
                oT[hi] = ps.tile([P, 512], F32, tag=f"oT{(len(oT)) % 2}",
                                 name=f"oT_{hi}", bufs=1)

                def pull():
                    try:
                        next(filler)
                    except StopIteration:
                        pass

                # iteration schedule: two-slot region one kt at a time;
                # single-slot region (kt >= Llo) in fused kt pairs
                iters = [(kt, kt) for kt in range(Llo)]
                iters += [(kt, kt + 1) for kt in range(Llo, Lhi, 2)]

                def score(it):
                    kt0, kt1 = it
                    sT2 = psS2()
                    pT = pp.tile([P, 2, 512], BF16, tag="pT", name="pT")
                    if kt0 == kt1:          # both slots at kt0
                        kt = kt0
                        cl = c0_of(lo, kt)
                        ch = c0_of(hi, kt)
                        assert ch == 0
                        nc.tensor.matmul(
                            sT2[:, 0, cl:], lhsT=kT[:, kt * P:(kt + 1) * P],
                            rhs=qT[:, q0 + cl:q0 + 512], start=True, stop=True,
                        )
                        nc.tensor.matmul(
                            sT2[:, 1, :], lhsT=kT[:, kt * P:(kt + 1) * P],
                            rhs=qT[:, q0 + 512:q0 + 1024], start=True, stop=True,
                        )
                        flat = sT2.rearrange("p a w -> p (a w)")
                        pf = pT.rearrange("p a w -> p (a w)")
                        nc.scalar.activation(pf[:, cl:], flat[:, cl:], Exp, scale=SCALE)
                        # mask windows are disjoint: lo in [0,Llo), hi in [Lhi-8,Lhi)
                        if kt >= Llo - 8:
                            # only the subtile at cl straddles the diagonal
                            nc.vector.scalar_tensor_tensor(
                                pT[:, 0, cl:cl + P], qpos_b[:, q0 + cl:q0 + cl + P],
                                kio_sb[:, kt:kt + 1], pT[:, 0, cl:cl + P],
                                op0=mybir.AluOpType.is_ge, op1=mybir.AluOpType.mult,
                            )
                        if kt == 0:
                            nc.vector.tensor_copy(dacc[:, 0:1024], pf[:])
                        else:
                            nc.vector.tensor_add(dacc[:, cl:1024], dacc[:, cl:1024],
                                                 pf[:, cl:])
                        return (pT, kt0, kt1, cl)
                    else:                   # hi only, fused kt pair
                        c0 = c0_of(hi, kt0)
                        c1 = c0_of(hi, kt1)
                        assert c0 == c1
                        for i, kt in enumerate((kt0, kt1)):
                            nc.tensor.matmul(
                                sT2[:, i, c0:], lhsT=kT[:, kt * P:(kt + 1) * P],
                                rhs=qT[:, q0 + 512 + c0:q0 + 1024],
                                start=True, stop=True,
                            )
                        nc.scalar.activation(pT[:, :, c0:], sT2[:, :, c0:], Exp,
                                             scale=SCALE)
                        if kt1 >= Lhi - 8:
                            for i, kt in enumerate((kt0, kt1)):
                                nc.vector.scalar_tensor_tensor(
                                    pT[:, i, c0:c0 + P],
                                    qpos_b[:, q0 + 512 + c0:q0 + 512 + c0 + P],
                                    kio_sb[:, kt:kt + 1], pT[:, i, c0:c0 + P],
                                    op0=mybir.AluOpType.is_ge, op1=mybir.AluOpType.mult,
                                )
                        nc.vector.tensor_add(
                            dacc[:, 512 + c0:1024], dacc[:, 512 + c0:1024],
                            pT[:, 0, c0:])
                        if kt0 == Llo:
                            nc.vector.tensor_copy(dacc[:, 1024:1536], pT[:, 1, :])
                        else:
                            nc.vector.tensor_add(
                                dacc[:, 1024 + c0:1536], dacc[:, 1024 + c0:1536],
                                pT[:, 1, c0:])
                        return (pT, kt0, kt1, None)

                def accum(st):
                    pT, kt0, kt1, cl = st
                    if kt0 == kt1:
                        kt = kt0
                        if kt < Llo:
                            nc.tensor.matmul(
                                oT[lo][:, cl:], lhsT=vN[:, kt, :], rhs=pT[:, 0, cl:],
                                start=(kt == 0), stop=(kt == Llo - 1),
                            )
                        nc.tensor.matmul(
                            oT[hi][:], lhsT=vN[:, kt, :], rhs=pT[:, 1, :],
                            start=(kt == 0), stop=(kt == Lhi - 1),
                        )
                    else:
                        c0 = c0_of(hi, kt0)
                        for i, kt in enumerate((kt0, kt1)):
                            nc.tensor.matmul(
                                oT[hi][:, c0:], lhsT=vN[:, kt, :], rhs=pT[:, i, c0:],
                                start=(kt == 0), stop=(kt == Lhi - 1),
                            )

                window = []
                for n, it in enumerate(iters):
                    window.append(score(it))
                    if n >= 4:
                        for _ in range(rate):
                            pull()
                    if len(window) > 3:
                        accum(window.pop(0))
                        if n >= 4:
                            for _ in range(rate):
                                pull()
                while window:
                    accum(window.pop(0))
                    pull()
                for _ in range(6):
                    pull()

            # ---------------- emission ----------------
            def chain(*gens):
                for g in gens:
                    yield from g

            def drain(g):
                for _ in g:
                    pass

            # pre-warm the ACT exp table during P-phase 0
            warm = epi.tile([P, 1], F32, tag="warm", name="warm")
            nc.scalar.activation(warm[:], ones_sb[:], Exp)

            # P0 in full (pass A needs vN bricks 0..7 early), but only the
            # K/Q parts of P1 gate pass A: its V rides as filler
            drain(phase_steps(0))
            drain(kq_steps(1))

            dacc_a = accp.tile([P, 1536], BF16, tag="dacc_a")
            dacc_b = accp.tile([P, 1536], BF16, tag="dacc_b")

            # P1's V plus the K/Q parts of P2/P3 ride as PE filler under
            # pass A (copies on DVE so the exp-critical ACT queue stays
            # clear).  P2/P3's V is deferred into pass B: vN bricks 16..31
            # are first consumed at kt>=16, and pass B is ACT-bound with
            # spare PE cycles.
            attention_pass(PASS_A, dacc_a,
                           chain(v_steps(1, False), kq_steps(2, False),
                                 kq_steps(3, False)), rate=3)

            # emit d/reciprocal/O-copy of slots 0,1 BEFORE pass B: pass B's
            # first accumulations WAR-wait on the oT ring slots these release,
            # so they must be ahead of pass B's DVE-queue ops (deadlock
            # avoidance), and they free the PSUM oT banks early.
            e0 = epilogue_steps(0, dacc_a, 0)
            e1 = epilogue_steps(1, dacc_a, 1, True)
            next(e0), next(e0)
            next(e1), next(e1)

            def skip(k):
                for _ in range(k):
                    yield None

            # slot 2's last accumulation pops at emission iter 26; spacers
            # keep its epilogue from being emitted before that
            attention_pass(PASS_B, dacc_b,
                           chain(v_steps(2, False), v_steps(3, False), e0, e1,
                                 skip(29), epilogue_steps(2, dacc_b, 0)))
            drain(epilogue_steps(3, dacc_b, 1, True, split=True))

    nc.compile()
    return nc


_NC_CACHE = None


def _get_nc():
    global _NC_CACHE
    if _NC_CACHE is None:
        _NC_CACHE = build_nc()
    return _NC_CACHE


def make_in_maps(inputs):
    x = np.asarray(inputs["x"], np.float32)
    Wq = np.asarray(inputs["Wq"], np.float32)
    Wk = np.asarray(inputs["Wk"], np.float32)
    Wv = np.asarray(inputs["Wv"], np.float32)
    bq = np.asarray(inputs["bq"], np.float32)

    xb = x.astype(NPBF16)

    def wshuf(W):
        # [D,H] -> [P, DCH*H]: partition p holds chunks c at rows c*128+p
        return np.ascontiguousarray(
            W.astype(NPBF16).reshape(DCH, P, H).transpose(1, 0, 2).reshape(P, DCH * H))

    common = dict(
        wq=wshuf(Wq), wk=wshuf(Wk), wv=wshuf(Wv),
        bq=bq.reshape(H, 1),
        identb=np.eye(P, dtype=NPBF16),
        onesb=np.ones((P, 1), dtype=NPBF16),
    )
    in_maps = []
    xbT = np.ascontiguousarray(xb.transpose(0, 2, 1))  # [B, D, S]
    swap = np.arange(NKT).reshape(-1, 2)[:, ::-1].reshape(-1)  # pair-swap blocks
    for c in range(8):
        b, hh = c // 2, c % 2
        m = dict(common)
        if hh == 0:
            m["xt"] = xbT[b]
            blk = np.arange(NKT)
        else:
            m["xt"] = np.ascontiguousarray(
                xbT[b].reshape(D, NKT, P)[:, swap, :].reshape(D, S))
            blk = swap
        # kio: global positions of local k-brick kt
        m["kio"] = (blk[None, :] * P + np.arange(P)[:, None] - P * hh).astype(np.int16)
        in_maps.append(m)
    return in_maps


def assemble_out(results):
    out = np.zeros((1, B, S, H), np.float32)
    for c in range(8):
        b, hh = c // 2, c % 2
        qg = qglob_for_core(hh)
        out[0, b, qg, :] = results[c]["out"].astype(np.float32)
    return out


def kernel(**inputs) -> np.ndarray:
    nc = _get_nc()
    in_maps = make_in_maps(inputs)
    res = run_bass_kernel_spmd(nc, in_maps, list(range(8)))
    out = assemble_out(res.results)
    bv = np.asarray(inputs["bv"], np.float32)
    out += (bv * SCALE)[None, None, None, :]
    return out
